# revision 9
# baseline (speedup 1.0000x reference)
"""Trainium2 Bass kernel for the 2-layer dependency-relation GCN (8 cores).

Math per layer l, token i:
    out[i] = relu( W_self[l] @ x[i] + b_self[l]
                   + sum_{e: dep[e]==i} (W_rel[l, rel[e]]   @ x[gov[e]] + b_rel[l, rel[e]])
                   + sum_{e: gov[e]==i} (W_rel[l, R+rel[e]] @ x[dep[e]] + b_rel[l, R+rel[e]]) )
final:  y = h @ W_ff.T + b_ff

The reference computes 40 dense [8192,512]x[512,512] GEMMs per layer; each
edge uses exactly one relation, so this kernel instead groups the 2N=16384
messages by relation and transforms only gathered source rows (13x fewer
FLOPs), relation-sharded across the 8 NeuronCores.

Pipeline per layer, per core c (owner of relations 5c..5c+4):
  1. transposing dma_gather of message sources (relation-grouped, padded to
     128-row M-tiles), chunked 256 idxs/instruction.
  2. per-tile GEMMs with SBUF-resident relation weights; PSUM -> bf16 rows
     are collected 4 tiles at a time into an SBUF staging tile.
  3. dma_scatter_add writes each staging group straight into the AllToAll
     send image (zero-filled per layer), laid out [half][peer][sub][R1 pad];
     GEMM pad rows go to a trash region past the wire window.  No message
     bounce buffer and no pack gather.
  4. two AllToAlls (sub-halves): the first half's accumulation overlaps the
     second half's wire time.
  5. per dest sub-block: ONE plain strided DMA pulls its [8 senders x R1]
     rows from the recv buffer; host-built one-hot matmuls scatter-add into
     PSUM; a DVE pass adds the precomputed self+bias tile; fused ReLU.
     Self+bias GEMMs run early (during gathers / collectives); layer 1's are
     emitted before the AllGather (via transposing gathers of h_own - XBAR
     transposes serialize against collectives) to fill its dead time.
final FF layer as before.
Numerics: bf16 matmul inputs / wire, fp32 PSUM accumulation.
"""

import numpy as np
import ml_dtypes

import concourse.bass as bass
import concourse.mybir as mybir
import concourse.tile as tile
from concourse import bacc
from concourse.bass_utils import run_bass_kernel_spmd

N = 8192
D = 512
R = 20
TWO_R = 2 * R
L = 2
OUT = 256
P = 128
NCORES = 8
RPC = TWO_R // NCORES    # 5 relations per core
BLK = N // NCORES        # 1024 tokens per core
NSUB = BLK // P          # 8 sub-blocks of 128 tokens
HSUB = NSUB // 2
KC = D // P              # 4 contraction chunks
CH_IDX = 2 * P           # idxs per transposing-gather chunk
SC_TILES = 4             # GEMM tiles per scatter-add group

BF16 = ml_dtypes.bfloat16

LAST_EXEC_TIME_NS = None
LAST_RESULTS = None

_CACHE = {}


def _pack_idx16(idx: np.ndarray) -> np.ndarray:
    Ln = len(idx)
    assert Ln % 16 == 0
    base = idx.astype(np.int16).reshape(Ln // 16, 16).T
    return np.tile(base, (8, 1)).copy()


def _plan(dep_idx: np.ndarray, rel_idx: np.ndarray, gov_idx: np.ndarray):
    dep = dep_idx.astype(np.int64)
    gov = gov_idx.astype(np.int64)
    rel = rel_idx.astype(np.int64)

    dest = np.concatenate([dep, gov])
    src = np.concatenate([gov, dep])
    r2 = np.concatenate([rel, rel + R])

    owner = r2 // RPC
    peer = dest // BLK
    sub = (dest % BLK) // P
    half = sub // HSUB
    ksub = sub % HSUB                 # sub index within the half

    # GEMM tiling: tiles per relation-slot, max over cores
    tps = []
    for s in range(RPC):
        mx = 1
        for c in range(NCORES):
            n = int((r2 == c * RPC + s).sum())
            mx = max(mx, int(np.ceil(n / P)))
        tps.append(mx)
    MT = sum(tps)
    tile_slot = []
    slot_tile_off = []
    off = 0
    for s in range(RPC):
        slot_tile_off.append(off)
        tile_slot.extend([s] * tps[s])
        off += tps[s]
    NMSG = MT * P

    # wire layout per half-segment for peer p:
    #   [ksub 0..3][R1M rows each] ++ [overflow block: OVER rows]
    # Cells larger than R1M spill into the per-(owner,peer,half) overflow
    # block; the receiver one-hots the shared overflow block per sub-block.
    cnt = np.zeros((NCORES, NCORES, NSUB), dtype=np.int64)
    for m in range(2 * N):
        cnt[owner[m], peer[m], sub[m]] += 1

    def over_for(r1m):
        ov = 0
        for c in range(NCORES):
            for p in range(NCORES):
                for hh in range(2):
                    tot = sum(
                        max(0, int(cnt[c, p, hh * HSUB + kl]) - r1m)
                        for kl in range(HSUB)
                    )
                    ov = max(ov, tot)
        return int(np.ceil(ov / 16) * 16) if ov else 0

    best = None
    for r1m in (16, 32, 48, 64):
        ov = over_for(r1m)
        seg = HSUB * r1m + ov
        # cost: wire bytes (seg) primary; one-hot matmul chunks secondary
        # (overflow chunks are one-hot'd once per sub-block -> 4x weight)
        chunks = (NCORES * r1m) // P + (NCORES * ov) // P
        key = (seg, chunks)
        if best is None or key < best[0]:
            best = (key, r1m, ov, seg)
    _, R1M, OVER, SEG = best
    if OVER == 0:
        OVER = 16
        SEG = HSUB * R1M + OVER
    SENDH = NCORES * SEG              # wire rows per half per rank
    J2M = NCORES * R1M // P
    J2O = NCORES * OVER // P
    assert (NCORES * R1M) % P == 0 and (NCORES * OVER) % P == 0
    NCHUNK = NSUB * (J2M + J2O)

    # per-message assignment
    msg_row = np.zeros(2 * N, dtype=np.int64)     # GEMM-output row (owner)
    send_slot = np.zeros(2 * N, dtype=np.int64)   # row in send image (owner)
    of_pos = np.zeros(2 * N, dtype=np.int64) - 1  # pos in overflow block
    for c in range(NCORES):
        cm = np.nonzero(owner == c)[0]
        fill = np.zeros(RPC, dtype=np.int64)
        rfill = np.zeros((NCORES, NSUB), dtype=np.int64)
        ofill = np.zeros((NCORES, 2), dtype=np.int64)
        for m in cm:
            sl = r2[m] - c * RPC
            msg_row[m] = slot_tile_off[sl] * P + fill[sl]
            fill[sl] += 1
            p = peer[m]
            hh = half[m]
            pos = rfill[p, sub[m]]
            rfill[p, sub[m]] += 1
            base = hh * SENDH + p * SEG
            if pos < R1M:
                send_slot[m] = base + ksub[m] * R1M + pos
            else:
                op_ = ofill[p, hh]
                assert op_ < OVER
                ofill[p, hh] += 1
                of_pos[m] = op_
                send_slot[m] = base + HSUB * R1M + op_
    TRASH = NMSG                      # worst case: every GEMM row is a pad
    SENDALL = 2 * SENDH + TRASH

    cores = []
    for c in range(NCORES):
        cm = np.nonzero(owner == c)[0]
        idxA = np.zeros(NMSG, dtype=np.int64)
        idxA[msg_row[cm]] = src[cm]
        # layer-1 source positions in the split-AllGather h_full layout:
        # token t lives at (t//BLK)*(BLK//2) + (t%BLK)  [+ N//2 if in the
        # upper half of its block]
        t = idxA
        lower = (t % BLK) < (BLK // 2)
        idxA2 = np.where(
            lower,
            (t // BLK) * (BLK // 2) + (t % BLK),
            N // 2 + (t // BLK) * (BLK // 2) + (t % BLK) - (BLK // 2),
        )

        # scatter slots in GEMM-row order; pads go to unique trash rows
        idxS = np.zeros(NMSG, dtype=np.int64)
        idxS[:] = 2 * SENDH + np.arange(NMSG)     # default: trash
        idxS[msg_row[cm]] = send_slot[cm]

        # one-hot matrices against the strided recv-load layout.
        # main load for sub k: rows rr = s*R1M + pos -> (partition rr//J2M,
        # chunk rr%J2M).  overflow load (per half, shared by its subs):
        # rows rr2 = s*OVER + ofpos -> (partition rr2//J2O, chunk rr2%J2O).
        S = np.zeros((NSUB, J2M + J2O, P, P), dtype=np.float32)
        dm = np.nonzero(peer == c)[0]
        for m in dm:
            k = sub[m]
            d = (dest[m] - c * BLK) % P
            if of_pos[m] < 0:
                pos = send_slot[m] - half[m] * SENDH - c * SEG - ksub[m] * R1M
                rr = owner[m] * R1M + pos
                S[k, rr % J2M, rr // J2M, d] = 1.0
            else:
                rr2 = owner[m] * OVER + of_pos[m]
                S[k, J2M + rr2 % J2O, rr2 // J2O, d] = 1.0

        CT = np.zeros((1 + TWO_R, BLK), dtype=np.float32)
        CT[0, :] = 1.0
        for m in dm:
            CT[1 + r2[m], dest[m] - c * BLK] += 1.0

        cores.append(
            dict(
                idxA=_pack_idx16(idxA),
                idxA2=_pack_idx16(idxA2),
                idxS=_pack_idx16(idxS),
                S=S.reshape(NSUB * (J2M + J2O) * P, P).astype(BF16),
                CT=CT.astype(BF16),
            )
        )

    return dict(
        MT=MT, tile_slot=tile_slot, NMSG=NMSG, R1M=R1M, OVER=OVER, SEG=SEG,
        SENDH=SENDH, SENDALL=SENDALL, J2M=J2M, J2O=J2O, NCHUNK=NCHUNK,
        cores=cores,
    )


def _build(MT, tile_slot, NMSG, R1M, OVER, SEG, SENDH, SENDALL, J2M, J2O, NCHUNK):
    nc = bacc.Bacc(
        "TRN2",
        target_bir_lowering=False,
        debug=False,
        enable_asserts=True,
        num_devices=NCORES,
    )
    dt = mybir.dt

    x0 = nc.dram_tensor("x0", [N, D], dt.bfloat16, kind="ExternalInput")
    x_own = nc.dram_tensor("x_own", [BLK, D], dt.bfloat16, kind="ExternalInput")
    wrel = nc.dram_tensor("wrel", [L, RPC, D, D], dt.bfloat16, kind="ExternalInput")
    wselfT = nc.dram_tensor("wselfT", [L, D, D], dt.bfloat16, kind="ExternalInput")
    bias = nc.dram_tensor("bias", [L, 1 + TWO_R, D], dt.bfloat16, kind="ExternalInput")
    ct = nc.dram_tensor("ct", [1 + TWO_R, BLK], dt.bfloat16, kind="ExternalInput")
    wffT = nc.dram_tensor("wffT", [D, OUT], dt.bfloat16, kind="ExternalInput")
    bff = nc.dram_tensor("bff", [1, OUT], dt.bfloat16, kind="ExternalInput")
    idxA = nc.dram_tensor("idxA", [P, NMSG // 16], dt.int16, kind="ExternalInput")
    idxA2 = nc.dram_tensor("idxA2", [P, NMSG // 16], dt.int16, kind="ExternalInput")
    idxS = nc.dram_tensor("idxS", [P, NMSG // 16], dt.int16, kind="ExternalInput")
    idxQ = nc.dram_tensor("idxQ", [P, BLK // 16], dt.int16, kind="ExternalInput")
    s_in = nc.dram_tensor("s", [NCHUNK * P, P], dt.bfloat16, kind="ExternalInput")
    y = nc.dram_tensor("y", [BLK, OUT], dt.float32, kind="ExternalOutput")

    h_own = nc.dram_tensor("h_own", [BLK, D], dt.bfloat16)
    h_full = nc.dram_tensor("h_full", [N, D], dt.bfloat16, addr_space="Shared")
    h2_own = nc.dram_tensor("h2_own", [BLK, D], dt.bfloat16)
    send_all = nc.dram_tensor("send_all", [SENDALL, D], dt.bfloat16)
    recv_bufs = [
        nc.dram_tensor(f"recv{hh}", [SENDH, D], dt.bfloat16) for hh in range(2)
    ]

    Relu = mybir.ActivationFunctionType.Relu

    with tile.TileContext(nc) as tc:
        with (
            tc.tile_pool(name="const", bufs=1) as const,
            tc.tile_pool(name="xtc", bufs=2) as xtcp,
            tc.tile_pool(name="xself", bufs=1) as xsp,
            tc.tile_pool(name="mso", bufs=2) as msop,
            tc.tile_pool(name="msgb", bufs=4) as msgbp,
            tc.tile_pool(name="selfb", bufs=8) as selfbp,
            tc.tile_pool(name="h", bufs=3) as hp,
            tc.tile_pool(name="psum_m", bufs=2, space="PSUM") as psum_m,
            tc.tile_pool(name="psum_o", bufs=4, space="PSUM") as psum_o,
            tc.tile_pool(name="psum_y", bufs=2, space="PSUM") as psum_y,
        ):
            # ---- constants; startup-critical loads first ----
            xself0 = xsp.tile([P, KC, BLK], dt.bfloat16, tag="xself")
            nc.sync.dma_start_transpose(xself0[:], x_own.ap())

            idxA_sb = const.tile([P, NMSG // 16], dt.int16)
            nc.sync.dma_start(idxA_sb[:], idxA.ap())
            idxA2_sb = const.tile([P, NMSG // 16], dt.int16)
            nc.scalar.dma_start(idxA2_sb[:], idxA2.ap())

            wselfT_sb = const.tile([P, L, KC, D], dt.bfloat16)
            nc.sync.dma_start(
                wselfT_sb[:], wselfT.ap().rearrange("l (c p) n -> p l c n", p=P)
            )
            ct_sb = const.tile([1 + TWO_R, BLK], dt.bfloat16)
            nc.scalar.dma_start(ct_sb[:], ct.ap())
            bias_sb = const.tile([1 + TWO_R, L, D], dt.bfloat16)
            nc.scalar.dma_start(bias_sb[:], bias.ap().rearrange("l b d -> b l d"))

            wrel_sb = [[None] * RPC for _ in range(L)]
            for ll in range(L):
                for ss in range(RPC):
                    wt = const.tile([P, KC, D], dt.bfloat16, tag=f"wrel{ll}_{ss}")
                    eng = nc.sync if (ll * RPC + ss) % 2 == 0 else nc.scalar
                    eng.dma_start(
                        wt[:], wrel.ap()[ll, ss].rearrange("(c p) n -> p c n", p=P)
                    )
                    wrel_sb[ll][ss] = wt

            idxS_sb = const.tile([P, NMSG // 16], dt.int16)
            idxQ_sb = const.tile([P, BLK // 16], dt.int16)
            nc.scalar.dma_start(idxS_sb[:], idxS.ap())
            nc.scalar.dma_start(idxQ_sb[:], idxQ.ap())
            s_sb = const.tile([P, NCHUNK, P], dt.bfloat16)
            nc.scalar.dma_start(s_sb[:], s_in.ap().rearrange("(c p) n -> p c n", p=P))
            wffT_sb = const.tile([P, KC, OUT], dt.bfloat16)
            nc.scalar.dma_start(wffT_sb[:], wffT.ap().rearrange("(c p) n -> p c n", p=P))
            bff_sb = const.tile([1, OUT], dt.bfloat16)
            nc.scalar.dma_start(bff_sb[:], bff.ap())
            ones_sb = const.tile([1, P], dt.bfloat16)
            nc.vector.memset(ones_sb[:], 1.0)
            zero_sb = const.tile([P, 8, D], dt.bfloat16)
            nc.vector.memset(zero_sb[:], 0.0)

            n_ch = (NMSG + CH_IDX - 1) // CH_IDX
            tiles_per_ch = CH_IDX // P
            n_grp = (MT + SC_TILES - 1) // SC_TILES

            def selfb_compute(layer, xs_of_k):
                tiles = []
                for k in range(NSUB):
                    xt, off = xs_of_k(k)
                    pm = psum_m.tile([P, D], dt.float32, space="PSUM", tag="pmsg")
                    for kc in range(KC):
                        nc.tensor.matmul(
                            out=pm[:],
                            lhsT=xt[:, kc, off : off + P],
                            rhs=wselfT_sb[:, layer, kc, :],
                            start=(kc == 0),
                            stop=False,
                        )
                    nc.tensor.matmul(
                        out=pm[:],
                        lhsT=ct_sb[:, k * P : (k + 1) * P],
                        rhs=bias_sb[:, layer, :],
                        start=False,
                        stop=True,
                    )
                    sb = selfbp.tile([P, D], dt.float32, tag="selfb")
                    nc.vector.tensor_copy(sb[:], pm[:])
                    tiles.append(sb)
                return tiles

            def zero_wire():
                # zero the wire region [0 : 2*SENDH) of the send image
                rows = 2 * SENDH
                zrows = P * 8
                for lo in range(0, rows, zrows):
                    hi = min(lo + zrows, rows)
                    nc.sync.dma_start(
                        send_all.ap()[lo:hi, :],
                        zero_sb[:, : (hi - lo) // P, :],
                    )

            def msg_phase(layer, src, idx_sb):
                grp_tile = None
                for ci in range(n_ch):
                    lo = ci * CH_IDX
                    hi = min(lo + CH_IDX, NMSG)
                    xc = xtcp.tile([P, KC, hi - lo], dt.bfloat16, tag="xTc")
                    nc.gpsimd.dma_gather(
                        out_ap=xc[:],
                        in_ap=src.ap(),
                        idxs_ap=idx_sb[:, lo // 16 : hi // 16],
                        num_idxs=hi - lo,
                        num_idxs_reg=hi - lo,
                        elem_size=D,
                        transpose=True,
                    )
                    for ti in range((hi - lo) // P):
                        mt = ci * tiles_per_ch + ti
                        g, gslot = divmod(mt, SC_TILES)
                        if gslot == 0:
                            grp_tile = msop.tile(
                                [P, SC_TILES, D], dt.bfloat16, tag="mso"
                            )
                        ss = tile_slot[mt]
                        pm = psum_m.tile([P, D], dt.float32, space="PSUM", tag="pmsg")
                        for kc in range(KC):
                            nc.tensor.matmul(
                                out=pm[:],
                                lhsT=xc[:, kc, ti * P : (ti + 1) * P],
                                rhs=wrel_sb[layer][ss][:, kc, :],
                                start=(kc == 0),
                                stop=(kc == KC - 1),
                            )
                        nc.vector.tensor_copy(grp_tile[:, gslot, :], pm[:])
                        if gslot == SC_TILES - 1 or mt == MT - 1:
                            nidx = (gslot + 1) * P
                            nc.gpsimd.dma_scatter_add(
                                send_all.ap(),
                                grp_tile[:, : gslot + 1, :],
                                idxS_sb[:, g * SC_TILES * P // 16 :
                                        (g * SC_TILES + gslot + 1) * P // 16],
                                nidx,
                                nidx,
                                D,
                            )

            def a2a(hh):
                nc.gpsimd.collective_compute(
                    "AllToAll",
                    mybir.AluOpType.bypass,
                    replica_groups=[list(range(NCORES))],
                    ins=[send_all.ap()[hh * SENDH : (hh + 1) * SENDH, :]],
                    outs=[recv_bufs[hh].ap()],
                )

            def accum_half(layer, hh, selfb, h_out, ff=False, h2T_tiles=None):
                seg = recv_bufs[hh].ap().rearrange("(s g) d -> s g d", s=NCORES)
                # eager loads on the scalar queue so they never sit behind
                # the h-writes of earlier sub-blocks
                ov = msgbp.tile([P, J2O, D], dt.bfloat16, tag="msgO")
                nc.scalar.dma_start(
                    ov[:], seg[:, HSUB * R1M : HSUB * R1M + OVER, :]
                )
                mbs = []
                for kl in range(HSUB):
                    mb = msgbp.tile([P, J2M, D], dt.bfloat16, tag="msgB")
                    nc.scalar.dma_start(
                        mb[:], seg[:, kl * R1M : (kl + 1) * R1M, :]
                    )
                    mbs.append(mb)
                JT = J2M + J2O
                for kl in range(HSUB):
                    k = hh * HSUB + kl
                    mb = mbs[kl]
                    po = psum_o.tile([P, D], dt.float32, space="PSUM", tag="pout")
                    for j in range(J2O):
                        nc.tensor.matmul(
                            out=po[:],
                            lhsT=s_sb[:, k * JT + J2M + j, :],
                            rhs=ov[:, j, :],
                            start=(j == 0),
                            stop=False,
                        )
                    for j in range(J2M):
                        nc.tensor.matmul(
                            out=po[:],
                            lhsT=s_sb[:, k * JT + j, :],
                            rhs=mb[:, j, :],
                            start=False,
                            stop=(j == J2M - 1),
                        )
                    nc.vector.tensor_add(out=po[:], in0=po[:], in1=selfb[k][:])
                    hsb = hp.tile([P, D], dt.bfloat16, tag="hsb")
                    nc.vector.tensor_scalar(
                        hsb[:], po[:], 0.0, None, mybir.AluOpType.max
                    )
                    nc.sync.dma_start(h_out.ap()[k * P : (k + 1) * P, :], hsb[:])
                    if ff:
                        # final layer: fold the FF GEMM for this sub-block in
                        # right away (transposing gather of the 128 rows just
                        # written), overlapping FF with the accumulation.
                        xt = xsp.tile([P, KC, P], dt.bfloat16, tag="h2T")
                        nc.gpsimd.dma_gather(
                            out_ap=xt[:],
                            in_ap=h_out.ap(),
                            idxs_ap=idxQ_sb[:, k * P // 16 : (k + 1) * P // 16],
                            num_idxs=P,
                            num_idxs_reg=P,
                            elem_size=D,
                            transpose=True,
                        )
                        py_ = psum_y.tile([P, OUT], dt.float32, space="PSUM", tag="py")
                        for kc in range(KC):
                            nc.tensor.matmul(
                                out=py_[:],
                                lhsT=xt[:, kc, :],
                                rhs=wffT_sb[:, kc, :],
                                start=(kc == 0),
                                stop=False,
                            )
                        nc.tensor.matmul(
                            out=py_[:], lhsT=ones_sb[:], rhs=bff_sb[:],
                            start=False, stop=True,
                        )
                        ysb = hp.tile([P, OUT], dt.float32, tag="ysb")
                        nc.vector.tensor_copy(ysb[:], py_[:])
                        nc.sync.dma_start(y.ap()[k * P : (k + 1) * P, :], ysb[:])

            def run_layer(layer, src, idx_sb, selfb, h_out, ff=False,
                          ag_split=False):
                zero_wire()
                msg_phase(layer, src, idx_sb)
                a2a(0)
                a2a(1)
                accum_half(layer, 0, selfb, h_out, ff=ff)
                if ag_split:
                    # AllGather of the first token-half right away; overlaps
                    # the second half's accumulation.
                    nc.gpsimd.collective_compute(
                        "AllGather",
                        mybir.AluOpType.bypass,
                        replica_groups=[list(range(NCORES))],
                        ins=[h_own.ap()[: BLK // 2, :]],
                        outs=[h_full.ap()[: N // 2, :]],
                    )
                accum_half(layer, 1, selfb, h_out, ff=ff)
                if ag_split:
                    nc.gpsimd.collective_compute(
                        "AllGather",
                        mybir.AluOpType.bypass,
                        replica_groups=[list(range(NCORES))],
                        ins=[h_own.ap()[BLK // 2 :, :]],
                        outs=[h_full.ap()[N // 2 :, :]],
                    )

            # tiny warm-up collectives: pay the ncfw/ring cold-start cost
            # during the (DMA-bound) startup instead of on layer 0's A2A.
            warm_in = nc.dram_tensor("warm_in", [16, 64], dt.bfloat16)
            warm_out = nc.dram_tensor("warm_out", [16, 64], dt.bfloat16)
            warm_ag = nc.dram_tensor(
                "warm_ag", [128, 64], dt.bfloat16, addr_space="Shared"
            )
            nc.sync.dma_start(warm_in.ap(), zero_sb[:16, 0, :64])
            nc.gpsimd.collective_compute(
                "AllToAll",
                mybir.AluOpType.bypass,
                replica_groups=[list(range(NCORES))],
                ins=[warm_in.ap()],
                outs=[warm_out.ap()],
            )
            nc.gpsimd.collective_compute(
                "AllGather",
                mybir.AluOpType.bypass,
                replica_groups=[list(range(NCORES))],
                ins=[warm_in.ap()],
                outs=[warm_ag.ap()],
            )
            nc.gpsimd.collective_compute(
                "AllToAll",
                mybir.AluOpType.bypass,
                replica_groups=[list(range(NCORES))],
                ins=[warm_in.ap()],
                outs=[warm_out.ap()],
            )

            # ================= layer 0 =================
            selfb0 = selfb_compute(0, lambda k: (xself0, k * P))
            run_layer(0, x0, idxA_sb, selfb0, h_own, ag_split=True)

            # layer-1 self work (fills the second AllGather's dead time)
            xq = []
            nxq = (BLK + CH_IDX - 1) // CH_IDX
            for ci in range(nxq):
                lo = ci * CH_IDX
                hi = min(lo + CH_IDX, BLK)
                xc = xsp.tile([P, KC, hi - lo], dt.bfloat16, tag=f"xq{ci}")
                nc.gpsimd.dma_gather(
                    out_ap=xc[:],
                    in_ap=h_own.ap(),
                    idxs_ap=idxQ_sb[:, lo // 16 : hi // 16],
                    num_idxs=hi - lo,
                    num_idxs_reg=hi - lo,
                    elem_size=D,
                    transpose=True,
                )
                xq.append(xc)
            selfb1 = selfb_compute(
                1, lambda k: (xq[(k * P) // CH_IDX], k * P - ((k * P) // CH_IDX) * CH_IDX)
            )

            # ================= layer 1 (with fused FF) =================
            run_layer(1, h_full, idxA2_sb, selfb1, h2_own, ff=True)

    nc.compile()
    return nc


def _in_maps(plan, x, W_self, b_self, W_rel, b_rel, W_ff, b_ff):
    x0 = x.astype(BF16)
    wselfT = np.ascontiguousarray(W_self.transpose(0, 2, 1)).astype(BF16)
    bias = np.concatenate([b_self[:, None, :], b_rel], axis=1).astype(BF16)
    wffT = np.ascontiguousarray(W_ff.T).astype(BF16)
    bffr = b_ff.reshape(1, OUT).astype(BF16)
    wrelT_all = np.ascontiguousarray(W_rel.transpose(0, 1, 3, 2)).astype(BF16)
    idxQ = _pack_idx16(np.arange(BLK))

    in_maps = []
    for c in range(NCORES):
        t = plan["cores"][c]
        in_maps.append(
            {
                "x0": x0,
                "x_own": np.ascontiguousarray(x0[c * BLK : (c + 1) * BLK]),
                "wrel": np.ascontiguousarray(wrelT_all[:, c * RPC : (c + 1) * RPC]),
                "wselfT": wselfT,
                "bias": bias,
                "ct": t["CT"],
                "wffT": wffT,
                "bff": bffr,
                "idxA": t["idxA"],
                "idxA2": t["idxA2"],
                "idxS": t["idxS"],
                "idxQ": idxQ,
                "s": t["S"],
            }
        )
    return in_maps


def kernel(x, dep_idx, rel_idx, gov_idx, W_self, b_self, W_rel, b_rel, W_ff, b_ff):
    global LAST_EXEC_TIME_NS, LAST_RESULTS

    x = np.asarray(x)
    dep_idx = np.asarray(dep_idx)
    rel_idx = np.asarray(rel_idx)
    gov_idx = np.asarray(gov_idx)
    W_self = np.asarray(W_self)
    b_self = np.asarray(b_self)
    W_rel = np.asarray(W_rel)
    b_rel = np.asarray(b_rel)
    W_ff = np.asarray(W_ff)
    b_ff = np.asarray(b_ff)
    assert x.shape == (N, D) and W_rel.shape == (L, TWO_R, D, D)

    key = (dep_idx.tobytes(), rel_idx.tobytes(), gov_idx.tobytes())
    if key in _CACHE:
        nc, plan = _CACHE[key]
    else:
        plan = _plan(dep_idx, rel_idx, gov_idx)
        nc = _build(
            plan["MT"], plan["tile_slot"], plan["NMSG"], plan["R1M"], plan["OVER"],
            plan["SEG"], plan["SENDH"], plan["SENDALL"], plan["J2M"], plan["J2O"],
            plan["NCHUNK"],
        )
        _CACHE.clear()
        _CACHE[key] = (nc, plan)

    in_maps = _in_maps(plan, x, W_self, b_self, W_rel, b_rel, W_ff, b_ff)
    res = run_bass_kernel_spmd(nc, in_maps, list(range(NCORES)))
    LAST_EXEC_TIME_NS = res.exec_time_ns
    LAST_RESULTS = res
    out = np.concatenate([res.results[c]["y"] for c in range(NCORES)], axis=0)
    return out.astype(np.float32)


# revision 10
# speedup vs baseline: 1.0503x; 1.0503x over previous
"""Trainium2 Bass kernel for the 2-layer dependency-relation GCN (8 cores).

Math per layer l, token i:
    out[i] = relu( W_self[l] @ x[i] + b_self[l]
                   + sum_{e: dep[e]==i} (W_rel[l, rel[e]]   @ x[gov[e]] + b_rel[l, rel[e]])
                   + sum_{e: gov[e]==i} (W_rel[l, R+rel[e]] @ x[dep[e]] + b_rel[l, R+rel[e]]) )
final:  y = h @ W_ff.T + b_ff

The reference computes 40 dense [8192,512]x[512,512] GEMMs per layer; each
edge uses exactly one relation, so this kernel instead groups the 2N=16384
messages by relation and transforms only gathered source rows (13x fewer
FLOPs), relation-sharded across the 8 NeuronCores.

Pipeline per layer, per core c (owner of relations 5c..5c+4):
  1. transposing dma_gather of message sources (relation-grouped, padded to
     128-row M-tiles), chunked 256 idxs/instruction.
  2. per-tile GEMMs with SBUF-resident relation weights; PSUM -> bf16 rows
     are collected 4 tiles at a time into an SBUF staging tile.
  3. dma_scatter_add writes each staging group straight into the AllToAll
     send image (zero-filled per layer), laid out [half][peer][sub][R1 pad];
     GEMM pad rows go to a trash region past the wire window.  No message
     bounce buffer and no pack gather.
  4. two AllToAlls (sub-halves): the first half's accumulation overlaps the
     second half's wire time.
  5. per dest sub-block: ONE plain strided DMA pulls its [8 senders x R1]
     rows from the recv buffer; host-built one-hot matmuls scatter-add into
     PSUM; a DVE pass adds the precomputed self+bias tile; fused ReLU.
     Self+bias GEMMs run early (during gathers / collectives); layer 1's are
     emitted before the AllGather (via transposing gathers of h_own - XBAR
     transposes serialize against collectives) to fill its dead time.
final FF layer as before.
Numerics: bf16 matmul inputs / wire, fp32 PSUM accumulation.
"""

import numpy as np
import ml_dtypes

import concourse.bass as bass
import concourse.mybir as mybir
import concourse.tile as tile
from concourse import bacc
from concourse.bass_utils import run_bass_kernel_spmd

N = 8192
D = 512
R = 20
TWO_R = 2 * R
L = 2
OUT = 256
P = 128
NCORES = 8
RPC = TWO_R // NCORES    # 5 relations per core
BLK = N // NCORES        # 1024 tokens per core
NSUB = BLK // P          # 8 sub-blocks of 128 tokens
HSUB = NSUB // 2
KC = D // P              # 4 contraction chunks
CH_IDX = 2 * P           # idxs per transposing-gather chunk
SC_TILES = 4             # GEMM tiles per scatter-add group

BF16 = ml_dtypes.bfloat16

LAST_EXEC_TIME_NS = None
LAST_RESULTS = None

_CACHE = {}


def _pack_idx16(idx: np.ndarray) -> np.ndarray:
    Ln = len(idx)
    assert Ln % 16 == 0
    base = idx.astype(np.int16).reshape(Ln // 16, 16).T
    return np.tile(base, (8, 1)).copy()


def _plan(dep_idx: np.ndarray, rel_idx: np.ndarray, gov_idx: np.ndarray):
    dep = dep_idx.astype(np.int64)
    gov = gov_idx.astype(np.int64)
    rel = rel_idx.astype(np.int64)

    dest = np.concatenate([dep, gov])
    src = np.concatenate([gov, dep])
    r2 = np.concatenate([rel, rel + R])

    owner = r2 // RPC
    peer = dest // BLK
    sub = (dest % BLK) // P
    half = sub // HSUB
    ksub = sub % HSUB                 # sub index within the half

    # GEMM tiling: tiles per relation-slot, max over cores
    tps = []
    for s in range(RPC):
        mx = 1
        for c in range(NCORES):
            n = int((r2 == c * RPC + s).sum())
            mx = max(mx, int(np.ceil(n / P)))
        tps.append(mx)
    MT = sum(tps)
    tile_slot = []
    slot_tile_off = []
    off = 0
    for s in range(RPC):
        slot_tile_off.append(off)
        tile_slot.extend([s] * tps[s])
        off += tps[s]
    NMSG = MT * P

    # wire layout per half-segment for peer p:
    #   [ksub 0..3][R1M rows each] ++ [overflow block: OVER rows]
    # Cells larger than R1M spill into the per-(owner,peer,half) overflow
    # block; the receiver one-hots the shared overflow block per sub-block.
    cnt = np.zeros((NCORES, NCORES, NSUB), dtype=np.int64)
    for m in range(2 * N):
        cnt[owner[m], peer[m], sub[m]] += 1

    def over_for(r1m):
        ov = 0
        for c in range(NCORES):
            for p in range(NCORES):
                for hh in range(2):
                    tot = sum(
                        max(0, int(cnt[c, p, hh * HSUB + kl]) - r1m)
                        for kl in range(HSUB)
                    )
                    ov = max(ov, tot)
        return int(np.ceil(ov / 16) * 16) if ov else 0

    best = None
    for r1m in (16, 32, 48, 64):
        ov = over_for(r1m)
        seg = HSUB * r1m + ov
        # cost: wire bytes (seg) primary; one-hot matmul chunks secondary
        # (overflow chunks are one-hot'd once per sub-block -> 4x weight)
        chunks = (NCORES * r1m) // P + (NCORES * ov) // P
        key = (seg, chunks)
        if best is None or key < best[0]:
            best = (key, r1m, ov, seg)
    _, R1M, OVER, SEG = best
    if OVER == 0:
        OVER = 16
        SEG = HSUB * R1M + OVER
    SENDH = NCORES * SEG              # wire rows per half per rank
    J2M = NCORES * R1M // P
    J2O = NCORES * OVER // P
    assert (NCORES * R1M) % P == 0 and (NCORES * OVER) % P == 0
    NCHUNK = NSUB * (J2M + J2O)

    # per-message assignment
    msg_row = np.zeros(2 * N, dtype=np.int64)     # GEMM-output row (owner)
    send_slot = np.zeros(2 * N, dtype=np.int64)   # row in send image (owner)
    of_pos = np.zeros(2 * N, dtype=np.int64) - 1  # pos in overflow block
    for c in range(NCORES):
        cm = np.nonzero(owner == c)[0]
        fill = np.zeros(RPC, dtype=np.int64)
        rfill = np.zeros((NCORES, NSUB), dtype=np.int64)
        ofill = np.zeros((NCORES, 2), dtype=np.int64)
        for m in cm:
            sl = r2[m] - c * RPC
            msg_row[m] = slot_tile_off[sl] * P + fill[sl]
            fill[sl] += 1
            p = peer[m]
            hh = half[m]
            pos = rfill[p, sub[m]]
            rfill[p, sub[m]] += 1
            base = hh * SENDH + p * SEG
            if pos < R1M:
                send_slot[m] = base + ksub[m] * R1M + pos
            else:
                op_ = ofill[p, hh]
                assert op_ < OVER
                ofill[p, hh] += 1
                of_pos[m] = op_
                send_slot[m] = base + HSUB * R1M + op_
    TRASH = NMSG                      # worst case: every GEMM row is a pad
    SENDALL = 2 * SENDH + TRASH

    cores = []
    for c in range(NCORES):
        cm = np.nonzero(owner == c)[0]
        idxA = np.zeros(NMSG, dtype=np.int64)
        idxA[msg_row[cm]] = src[cm]
        # layer-1 source positions in the split-AllGather h_full layout:
        # token t lives at (t//BLK)*(BLK//2) + (t%BLK)  [+ N//2 if in the
        # upper half of its block]
        t = idxA
        lower = (t % BLK) < (BLK // 2)
        idxA2 = np.where(
            lower,
            (t // BLK) * (BLK // 2) + (t % BLK),
            N // 2 + (t // BLK) * (BLK // 2) + (t % BLK) - (BLK // 2),
        )

        # scatter slots in GEMM-row order; pads go to unique trash rows
        idxS = np.zeros(NMSG, dtype=np.int64)
        idxS[:] = 2 * SENDH + np.arange(NMSG)     # default: trash
        idxS[msg_row[cm]] = send_slot[cm]

        # one-hot matrices against the strided recv-load layout.
        # main load for sub k: rows rr = s*R1M + pos -> (partition rr//J2M,
        # chunk rr%J2M).  overflow load (per half, shared by its subs):
        # rows rr2 = s*OVER + ofpos -> (partition rr2//J2O, chunk rr2%J2O).
        S = np.zeros((NSUB, J2M + J2O, P, P), dtype=np.float32)
        dm = np.nonzero(peer == c)[0]
        for m in dm:
            k = sub[m]
            d = (dest[m] - c * BLK) % P
            if of_pos[m] < 0:
                pos = send_slot[m] - half[m] * SENDH - c * SEG - ksub[m] * R1M
                rr = owner[m] * R1M + pos
                S[k, rr % J2M, rr // J2M, d] = 1.0
            else:
                rr2 = owner[m] * OVER + of_pos[m]
                S[k, J2M + rr2 % J2O, rr2 // J2O, d] = 1.0

        CT = np.zeros((1 + TWO_R, BLK), dtype=np.float32)
        CT[0, :] = 1.0
        for m in dm:
            CT[1 + r2[m], dest[m] - c * BLK] += 1.0

        cores.append(
            dict(
                idxA=_pack_idx16(idxA),
                idxA2=_pack_idx16(idxA2),
                idxS=_pack_idx16(idxS),
                S=S.reshape(NSUB * (J2M + J2O) * P, P).astype(BF16),
                CT=CT.astype(BF16),
            )
        )

    return dict(
        MT=MT, tile_slot=tile_slot, NMSG=NMSG, R1M=R1M, OVER=OVER, SEG=SEG,
        SENDH=SENDH, SENDALL=SENDALL, J2M=J2M, J2O=J2O, NCHUNK=NCHUNK,
        cores=cores,
    )


def _build(MT, tile_slot, NMSG, R1M, OVER, SEG, SENDH, SENDALL, J2M, J2O, NCHUNK):
    nc = bacc.Bacc(
        "TRN2",
        target_bir_lowering=False,
        debug=False,
        enable_asserts=True,
        num_devices=NCORES,
    )
    dt = mybir.dt

    x0 = nc.dram_tensor("x0", [N, D], dt.bfloat16, kind="ExternalInput")
    x_own = nc.dram_tensor("x_own", [BLK, D], dt.bfloat16, kind="ExternalInput")
    wrel = nc.dram_tensor("wrel", [L, RPC, D, D], dt.bfloat16, kind="ExternalInput")
    wselfT = nc.dram_tensor("wselfT", [L, D, D], dt.bfloat16, kind="ExternalInput")
    bias = nc.dram_tensor("bias", [L, 1 + TWO_R, D], dt.bfloat16, kind="ExternalInput")
    ct = nc.dram_tensor("ct", [1 + TWO_R, BLK], dt.bfloat16, kind="ExternalInput")
    wffT = nc.dram_tensor("wffT", [D, OUT], dt.bfloat16, kind="ExternalInput")
    bff = nc.dram_tensor("bff", [1, OUT], dt.bfloat16, kind="ExternalInput")
    idxA = nc.dram_tensor("idxA", [P, NMSG // 16], dt.int16, kind="ExternalInput")
    idxA2 = nc.dram_tensor("idxA2", [P, NMSG // 16], dt.int16, kind="ExternalInput")
    idxS = nc.dram_tensor("idxS", [P, NMSG // 16], dt.int16, kind="ExternalInput")
    idxQ = nc.dram_tensor("idxQ", [P, BLK // 16], dt.int16, kind="ExternalInput")
    s_in = nc.dram_tensor("s", [NCHUNK * P, P], dt.bfloat16, kind="ExternalInput")
    y = nc.dram_tensor("y", [BLK, OUT], dt.float32, kind="ExternalOutput")

    h_own = nc.dram_tensor("h_own", [BLK, D], dt.bfloat16)
    h_full = nc.dram_tensor("h_full", [N, D], dt.bfloat16, addr_space="Shared")
    h2_own = nc.dram_tensor("h2_own", [BLK, D], dt.bfloat16)
    send_all = nc.dram_tensor("send_all", [SENDALL, D], dt.bfloat16)
    recv_bufs = [
        nc.dram_tensor(f"recv{hh}", [SENDH, D], dt.bfloat16) for hh in range(2)
    ]

    Relu = mybir.ActivationFunctionType.Relu

    with tile.TileContext(nc) as tc:
        with (
            tc.tile_pool(name="const", bufs=1) as const,
            tc.tile_pool(name="xtc", bufs=2) as xtcp,
            tc.tile_pool(name="xself", bufs=1) as xsp,
            tc.tile_pool(name="mso", bufs=2) as msop,
            tc.tile_pool(name="msgb", bufs=4) as msgbp,
            tc.tile_pool(name="selfb", bufs=8) as selfbp,
            tc.tile_pool(name="h", bufs=3) as hp,
            tc.tile_pool(name="psum_m", bufs=2, space="PSUM") as psum_m,
            tc.tile_pool(name="psum_o", bufs=4, space="PSUM") as psum_o,
            tc.tile_pool(name="psum_y", bufs=2, space="PSUM") as psum_y,
        ):
            # ---- constants; startup-critical loads first ----
            xself0 = xsp.tile([P, KC, BLK], dt.bfloat16, tag="xself")
            nc.sync.dma_start_transpose(xself0[:], x_own.ap())

            idxA_sb = const.tile([P, NMSG // 16], dt.int16)
            nc.sync.dma_start(idxA_sb[:], idxA.ap())
            idxA2_sb = const.tile([P, NMSG // 16], dt.int16)
            nc.scalar.dma_start(idxA2_sb[:], idxA2.ap())

            wselfT_sb = const.tile([P, L, KC, D], dt.bfloat16)
            nc.sync.dma_start(
                wselfT_sb[:], wselfT.ap().rearrange("l (c p) n -> p l c n", p=P)
            )
            ct_sb = const.tile([1 + TWO_R, BLK], dt.bfloat16)
            nc.scalar.dma_start(ct_sb[:], ct.ap())
            bias_sb = const.tile([1 + TWO_R, L, D], dt.bfloat16)
            nc.scalar.dma_start(bias_sb[:], bias.ap().rearrange("l b d -> b l d"))

            wrel_sb = [[None] * RPC for _ in range(L)]
            for ll in range(L):
                for ss in range(RPC):
                    wt = const.tile([P, KC, D], dt.bfloat16, tag=f"wrel{ll}_{ss}")
                    eng = nc.sync if (ll * RPC + ss) % 2 == 0 else nc.scalar
                    eng.dma_start(
                        wt[:], wrel.ap()[ll, ss].rearrange("(c p) n -> p c n", p=P)
                    )
                    wrel_sb[ll][ss] = wt

            idxS_sb = const.tile([P, NMSG // 16], dt.int16)
            idxQ_sb = const.tile([P, BLK // 16], dt.int16)
            nc.scalar.dma_start(idxS_sb[:], idxS.ap())
            nc.scalar.dma_start(idxQ_sb[:], idxQ.ap())
            s_sb = const.tile([P, NCHUNK, P], dt.bfloat16)
            nc.scalar.dma_start(s_sb[:], s_in.ap().rearrange("(c p) n -> p c n", p=P))
            wffT_sb = const.tile([P, KC, OUT], dt.bfloat16)
            nc.scalar.dma_start(wffT_sb[:], wffT.ap().rearrange("(c p) n -> p c n", p=P))
            bff_sb = const.tile([1, OUT], dt.bfloat16)
            nc.scalar.dma_start(bff_sb[:], bff.ap())
            ones_sb = const.tile([1, P], dt.bfloat16)
            nc.vector.memset(ones_sb[:], 1.0)
            zero_sb = const.tile([P, 8, D], dt.bfloat16)
            nc.vector.memset(zero_sb[:], 0.0)

            n_ch = (NMSG + CH_IDX - 1) // CH_IDX
            tiles_per_ch = CH_IDX // P
            n_grp = (MT + SC_TILES - 1) // SC_TILES

            def selfb_compute(layer, xs_of_k):
                tiles = []
                for k in range(NSUB):
                    xt, off = xs_of_k(k)
                    pm = psum_m.tile([P, D], dt.float32, space="PSUM", tag="pmsg")
                    for kc in range(KC):
                        nc.tensor.matmul(
                            out=pm[:],
                            lhsT=xt[:, kc, off : off + P],
                            rhs=wselfT_sb[:, layer, kc, :],
                            start=(kc == 0),
                            stop=False,
                        )
                    nc.tensor.matmul(
                        out=pm[:],
                        lhsT=ct_sb[:, k * P : (k + 1) * P],
                        rhs=bias_sb[:, layer, :],
                        start=False,
                        stop=True,
                    )
                    sb = selfbp.tile([P, D], dt.float32, tag="selfb")
                    nc.vector.tensor_copy(sb[:], pm[:])
                    tiles.append(sb)
                return tiles

            def zero_wire():
                # zero the wire region [0 : 2*SENDH) of the send image
                rows = 2 * SENDH
                zrows = P * 8
                for lo in range(0, rows, zrows):
                    hi = min(lo + zrows, rows)
                    nc.sync.dma_start(
                        send_all.ap()[lo:hi, :],
                        zero_sb[:, : (hi - lo) // P, :],
                    )

            def msg_phase(layer, src, idx_sb):
                grp_tile = None
                for ci in range(n_ch):
                    lo = ci * CH_IDX
                    hi = min(lo + CH_IDX, NMSG)
                    xc = xtcp.tile([P, KC, hi - lo], dt.bfloat16, tag="xTc")
                    nc.gpsimd.dma_gather(
                        out_ap=xc[:],
                        in_ap=src.ap(),
                        idxs_ap=idx_sb[:, lo // 16 : hi // 16],
                        num_idxs=hi - lo,
                        num_idxs_reg=hi - lo,
                        elem_size=D,
                        transpose=True,
                    )
                    for ti in range((hi - lo) // P):
                        mt = ci * tiles_per_ch + ti
                        g, gslot = divmod(mt, SC_TILES)
                        if gslot == 0:
                            grp_tile = msop.tile(
                                [P, SC_TILES, D], dt.bfloat16, tag="mso"
                            )
                        ss = tile_slot[mt]
                        pm = psum_m.tile([P, D], dt.float32, space="PSUM", tag="pmsg")
                        for kc in range(KC):
                            nc.tensor.matmul(
                                out=pm[:],
                                lhsT=xc[:, kc, ti * P : (ti + 1) * P],
                                rhs=wrel_sb[layer][ss][:, kc, :],
                                start=(kc == 0),
                                stop=(kc == KC - 1),
                            )
                        nc.vector.tensor_copy(grp_tile[:, gslot, :], pm[:])
                        if gslot == SC_TILES - 1 or mt == MT - 1:
                            nidx = (gslot + 1) * P
                            nc.gpsimd.dma_scatter_add(
                                send_all.ap(),
                                grp_tile[:, : gslot + 1, :],
                                idxS_sb[:, g * SC_TILES * P // 16 :
                                        (g * SC_TILES + gslot + 1) * P // 16],
                                nidx,
                                nidx,
                                D,
                            )

            def a2a(hh):
                nc.gpsimd.collective_compute(
                    "AllToAll",
                    mybir.AluOpType.bypass,
                    replica_groups=[list(range(NCORES))],
                    ins=[send_all.ap()[hh * SENDH : (hh + 1) * SENDH, :]],
                    outs=[recv_bufs[hh].ap()],
                )

            def accum_half(layer, hh, selfb, h_out, ff=False, h2T_tiles=None):
                seg = recv_bufs[hh].ap().rearrange("(s g) d -> s g d", s=NCORES)
                # eager loads on the scalar queue so they never sit behind
                # the h-writes of earlier sub-blocks
                ov = msgbp.tile([P, J2O, D], dt.bfloat16, tag="msgO")
                nc.scalar.dma_start(
                    ov[:], seg[:, HSUB * R1M : HSUB * R1M + OVER, :]
                )
                mbs = []
                for kl in range(HSUB):
                    mb = msgbp.tile([P, J2M, D], dt.bfloat16, tag="msgB")
                    nc.scalar.dma_start(
                        mb[:], seg[:, kl * R1M : (kl + 1) * R1M, :]
                    )
                    mbs.append(mb)
                JT = J2M + J2O
                for kl in range(HSUB):
                    k = hh * HSUB + kl
                    mb = mbs[kl]
                    po = psum_o.tile([P, D], dt.float32, space="PSUM", tag="pout")
                    for j in range(J2M):
                        nc.tensor.matmul(
                            out=po[:],
                            lhsT=s_sb[:, k * JT + j, :],
                            rhs=mb[:, j, :],
                            start=(j == 0),
                            stop=False,
                        )
                    for j in range(J2O):
                        nc.tensor.matmul(
                            out=po[:],
                            lhsT=s_sb[:, k * JT + J2M + j, :],
                            rhs=ov[:, j, :],
                            start=False,
                            stop=(j == J2O - 1),
                        )
                    nc.vector.tensor_add(out=po[:], in0=po[:], in1=selfb[k][:])
                    hsb = hp.tile([P, D], dt.bfloat16, tag="hsb")
                    nc.vector.tensor_scalar(
                        hsb[:], po[:], 0.0, None, mybir.AluOpType.max
                    )
                    nc.sync.dma_start(h_out.ap()[k * P : (k + 1) * P, :], hsb[:])
                    if ff:
                        # final layer: fold the FF GEMM for this sub-block in
                        # right away (transposing gather of the 128 rows just
                        # written), overlapping FF with the accumulation.
                        xt = xsp.tile([P, KC, P], dt.bfloat16, tag="h2T")
                        nc.gpsimd.dma_gather(
                            out_ap=xt[:],
                            in_ap=h_out.ap(),
                            idxs_ap=idxQ_sb[:, k * P // 16 : (k + 1) * P // 16],
                            num_idxs=P,
                            num_idxs_reg=P,
                            elem_size=D,
                            transpose=True,
                        )
                        py_ = psum_y.tile([P, OUT], dt.float32, space="PSUM", tag="py")
                        for kc in range(KC):
                            nc.tensor.matmul(
                                out=py_[:],
                                lhsT=xt[:, kc, :],
                                rhs=wffT_sb[:, kc, :],
                                start=(kc == 0),
                                stop=False,
                            )
                        nc.tensor.matmul(
                            out=py_[:], lhsT=ones_sb[:], rhs=bff_sb[:],
                            start=False, stop=True,
                        )
                        ysb = hp.tile([P, OUT], dt.float32, tag="ysb")
                        nc.vector.tensor_copy(ysb[:], py_[:])
                        nc.sync.dma_start(y.ap()[k * P : (k + 1) * P, :], ysb[:])

            def run_layer(layer, src, idx_sb, selfb, h_out, ff=False,
                          ag_split=False):
                zero_wire()
                msg_phase(layer, src, idx_sb)
                a2a(0)
                a2a(1)
                accum_half(layer, 0, selfb, h_out, ff=ff)
                if ag_split:
                    # AllGather of the first token-half right away; overlaps
                    # the second half's accumulation.
                    nc.gpsimd.collective_compute(
                        "AllGather",
                        mybir.AluOpType.bypass,
                        replica_groups=[list(range(NCORES))],
                        ins=[h_own.ap()[: BLK // 2, :]],
                        outs=[h_full.ap()[: N // 2, :]],
                    )
                accum_half(layer, 1, selfb, h_out, ff=ff)
                if ag_split:
                    nc.gpsimd.collective_compute(
                        "AllGather",
                        mybir.AluOpType.bypass,
                        replica_groups=[list(range(NCORES))],
                        ins=[h_own.ap()[BLK // 2 :, :]],
                        outs=[h_full.ap()[N // 2 :, :]],
                    )

            # tiny warm-up collectives: pay the ncfw/ring cold-start cost
            # during the (DMA-bound) startup instead of on layer 0's A2A.
            warm_in = nc.dram_tensor("warm_in", [16, 64], dt.bfloat16)
            warm_out = nc.dram_tensor("warm_out", [16, 64], dt.bfloat16)
            warm_ag = nc.dram_tensor(
                "warm_ag", [128, 64], dt.bfloat16, addr_space="Shared"
            )
            nc.sync.dma_start(warm_in.ap(), zero_sb[:16, 0, :64])
            nc.gpsimd.collective_compute(
                "AllToAll",
                mybir.AluOpType.bypass,
                replica_groups=[list(range(NCORES))],
                ins=[warm_in.ap()],
                outs=[warm_out.ap()],
            )
            nc.gpsimd.collective_compute(
                "AllGather",
                mybir.AluOpType.bypass,
                replica_groups=[list(range(NCORES))],
                ins=[warm_in.ap()],
                outs=[warm_ag.ap()],
            )

            # ================= layer 0 =================
            selfb0 = selfb_compute(0, lambda k: (xself0, k * P))
            run_layer(0, x0, idxA_sb, selfb0, h_own, ag_split=True)

            # layer-1 self work (fills the second AllGather's dead time)
            xq = []
            nxq = (BLK + CH_IDX - 1) // CH_IDX
            for ci in range(nxq):
                lo = ci * CH_IDX
                hi = min(lo + CH_IDX, BLK)
                xc = xsp.tile([P, KC, hi - lo], dt.bfloat16, tag=f"xq{ci}")
                nc.gpsimd.dma_gather(
                    out_ap=xc[:],
                    in_ap=h_own.ap(),
                    idxs_ap=idxQ_sb[:, lo // 16 : hi // 16],
                    num_idxs=hi - lo,
                    num_idxs_reg=hi - lo,
                    elem_size=D,
                    transpose=True,
                )
                xq.append(xc)
            selfb1 = selfb_compute(
                1, lambda k: (xq[(k * P) // CH_IDX], k * P - ((k * P) // CH_IDX) * CH_IDX)
            )

            # ================= layer 1 (with fused FF) =================
            run_layer(1, h_full, idxA2_sb, selfb1, h2_own, ff=True)

    nc.compile()
    return nc


def _in_maps(plan, x, W_self, b_self, W_rel, b_rel, W_ff, b_ff):
    x0 = x.astype(BF16)
    wselfT = np.ascontiguousarray(W_self.transpose(0, 2, 1)).astype(BF16)
    bias = np.concatenate([b_self[:, None, :], b_rel], axis=1).astype(BF16)
    wffT = np.ascontiguousarray(W_ff.T).astype(BF16)
    bffr = b_ff.reshape(1, OUT).astype(BF16)
    wrelT_all = np.ascontiguousarray(W_rel.transpose(0, 1, 3, 2)).astype(BF16)
    idxQ = _pack_idx16(np.arange(BLK))

    in_maps = []
    for c in range(NCORES):
        t = plan["cores"][c]
        in_maps.append(
            {
                "x0": x0,
                "x_own": np.ascontiguousarray(x0[c * BLK : (c + 1) * BLK]),
                "wrel": np.ascontiguousarray(wrelT_all[:, c * RPC : (c + 1) * RPC]),
                "wselfT": wselfT,
                "bias": bias,
                "ct": t["CT"],
                "wffT": wffT,
                "bff": bffr,
                "idxA": t["idxA"],
                "idxA2": t["idxA2"],
                "idxS": t["idxS"],
                "idxQ": idxQ,
                "s": t["S"],
            }
        )
    return in_maps


def kernel(x, dep_idx, rel_idx, gov_idx, W_self, b_self, W_rel, b_rel, W_ff, b_ff):
    global LAST_EXEC_TIME_NS, LAST_RESULTS

    x = np.asarray(x)
    dep_idx = np.asarray(dep_idx)
    rel_idx = np.asarray(rel_idx)
    gov_idx = np.asarray(gov_idx)
    W_self = np.asarray(W_self)
    b_self = np.asarray(b_self)
    W_rel = np.asarray(W_rel)
    b_rel = np.asarray(b_rel)
    W_ff = np.asarray(W_ff)
    b_ff = np.asarray(b_ff)
    assert x.shape == (N, D) and W_rel.shape == (L, TWO_R, D, D)

    key = (dep_idx.tobytes(), rel_idx.tobytes(), gov_idx.tobytes())
    if key in _CACHE:
        nc, plan = _CACHE[key]
    else:
        plan = _plan(dep_idx, rel_idx, gov_idx)
        nc = _build(
            plan["MT"], plan["tile_slot"], plan["NMSG"], plan["R1M"], plan["OVER"],
            plan["SEG"], plan["SENDH"], plan["SENDALL"], plan["J2M"], plan["J2O"],
            plan["NCHUNK"],
        )
        _CACHE.clear()
        _CACHE[key] = (nc, plan)

    in_maps = _in_maps(plan, x, W_self, b_self, W_rel, b_rel, W_ff, b_ff)
    res = run_bass_kernel_spmd(nc, in_maps, list(range(NCORES)))
    LAST_EXEC_TIME_NS = res.exec_time_ns
    LAST_RESULTS = res
    out = np.concatenate([res.results[c]["y"] for c in range(NCORES)], axis=0)
    return out.astype(np.float32)


# revision 11
# speedup vs baseline: 1.0614x; 1.0106x over previous
"""Trainium2 Bass kernel for the 2-layer dependency-relation GCN (8 cores).

Math per layer l, token i:
    out[i] = relu( W_self[l] @ x[i] + b_self[l]
                   + sum_{e: dep[e]==i} (W_rel[l, rel[e]]   @ x[gov[e]] + b_rel[l, rel[e]])
                   + sum_{e: gov[e]==i} (W_rel[l, R+rel[e]] @ x[dep[e]] + b_rel[l, R+rel[e]]) )
final:  y = h @ W_ff.T + b_ff

The reference computes 40 dense [8192,512]x[512,512] GEMMs per layer; each
edge uses exactly one relation, so this kernel instead groups the 2N=16384
messages by relation and transforms only gathered source rows (13x fewer
FLOPs), relation-sharded across the 8 NeuronCores.

Pipeline per layer, per core c (owner of relations 5c..5c+4):
  1. transposing dma_gather of message sources (relation-grouped, padded to
     128-row M-tiles), chunked 256 idxs/instruction.
  2. per-tile GEMMs with SBUF-resident relation weights; PSUM -> bf16 rows
     are collected 4 tiles at a time into an SBUF staging tile.
  3. dma_scatter_add writes each staging group straight into the AllToAll
     send image (zero-filled per layer), laid out [half][peer][sub][R1 pad];
     GEMM pad rows go to a trash region past the wire window.  No message
     bounce buffer and no pack gather.
  4. two AllToAlls (sub-halves): the first half's accumulation overlaps the
     second half's wire time.
  5. per dest sub-block: ONE plain strided DMA pulls its [8 senders x R1]
     rows from the recv buffer; host-built one-hot matmuls scatter-add into
     PSUM; a DVE pass adds the precomputed self+bias tile; fused ReLU.
     Self+bias GEMMs run early (during gathers / collectives); layer 1's are
     emitted before the AllGather (via transposing gathers of h_own - XBAR
     transposes serialize against collectives) to fill its dead time.
final FF layer as before.
Numerics: bf16 matmul inputs / wire, fp32 PSUM accumulation.
"""

import numpy as np
import ml_dtypes

import concourse.bass as bass
import concourse.mybir as mybir
import concourse.tile as tile
from concourse import bacc
from concourse.bass_utils import run_bass_kernel_spmd

N = 8192
D = 512
R = 20
TWO_R = 2 * R
L = 2
OUT = 256
P = 128
NCORES = 8
RPC = TWO_R // NCORES    # 5 relations per core
BLK = N // NCORES        # 1024 tokens per core
NSUB = BLK // P          # 8 sub-blocks of 128 tokens
HSUB = NSUB // 2
KC = D // P              # 4 contraction chunks
CH_IDX = 2 * P           # idxs per transposing-gather chunk
SC_TILES = 4             # GEMM tiles per scatter-add group

BF16 = ml_dtypes.bfloat16

LAST_EXEC_TIME_NS = None
LAST_RESULTS = None

_CACHE = {}


def _pack_idx16(idx: np.ndarray) -> np.ndarray:
    Ln = len(idx)
    assert Ln % 16 == 0
    base = idx.astype(np.int16).reshape(Ln // 16, 16).T
    return np.tile(base, (8, 1)).copy()


def _plan(dep_idx: np.ndarray, rel_idx: np.ndarray, gov_idx: np.ndarray):
    dep = dep_idx.astype(np.int64)
    gov = gov_idx.astype(np.int64)
    rel = rel_idx.astype(np.int64)

    dest = np.concatenate([dep, gov])
    src = np.concatenate([gov, dep])
    r2 = np.concatenate([rel, rel + R])

    owner = r2 // RPC
    peer = dest // BLK
    sub = (dest % BLK) // P
    half = sub // HSUB
    ksub = sub % HSUB                 # sub index within the half

    # GEMM tiling: tiles per relation-slot, max over cores
    tps = []
    for s in range(RPC):
        mx = 1
        for c in range(NCORES):
            n = int((r2 == c * RPC + s).sum())
            mx = max(mx, int(np.ceil(n / P)))
        tps.append(mx)
    MT = sum(tps)
    tile_slot = []
    slot_tile_off = []
    off = 0
    for s in range(RPC):
        slot_tile_off.append(off)
        tile_slot.extend([s] * tps[s])
        off += tps[s]
    NMSG = MT * P

    # wire layout per half-segment for peer p:
    #   [ksub 0..3][R1M rows each] ++ [overflow block: OVER rows]
    # Cells larger than R1M spill into the per-(owner,peer,half) overflow
    # block; the receiver one-hots the shared overflow block per sub-block.
    cnt = np.zeros((NCORES, NCORES, NSUB), dtype=np.int64)
    for m in range(2 * N):
        cnt[owner[m], peer[m], sub[m]] += 1

    def over_for(r1m):
        ov = 0
        for c in range(NCORES):
            for p in range(NCORES):
                for hh in range(2):
                    tot = sum(
                        max(0, int(cnt[c, p, hh * HSUB + kl]) - r1m)
                        for kl in range(HSUB)
                    )
                    ov = max(ov, tot)
        return int(np.ceil(ov / 16) * 16) if ov else 0

    best = None
    for r1m in (16, 32, 48, 64):
        ov = over_for(r1m)
        seg = HSUB * r1m + ov
        # cost: wire bytes (seg) primary; one-hot matmul chunks secondary
        # (overflow chunks are one-hot'd once per sub-block -> 4x weight)
        chunks = (NCORES * r1m) // P + (NCORES * ov) // P
        key = (seg, chunks)
        if best is None or key < best[0]:
            best = (key, r1m, ov, seg)
    _, R1M, OVER, SEG = best
    if OVER == 0:
        OVER = 16
        SEG = HSUB * R1M + OVER
    SENDH = NCORES * SEG              # wire rows per half per rank
    J2M = NCORES * R1M // P
    J2O = NCORES * OVER // P
    assert (NCORES * R1M) % P == 0 and (NCORES * OVER) % P == 0
    NCHUNK = NSUB * (J2M + J2O)

    # per-message assignment
    msg_row = np.zeros(2 * N, dtype=np.int64)     # GEMM-output row (owner)
    send_slot = np.zeros(2 * N, dtype=np.int64)   # row in send image (owner)
    of_pos = np.zeros(2 * N, dtype=np.int64) - 1  # pos in overflow block
    for c in range(NCORES):
        cm = np.nonzero(owner == c)[0]
        fill = np.zeros(RPC, dtype=np.int64)
        rfill = np.zeros((NCORES, NSUB), dtype=np.int64)
        ofill = np.zeros((NCORES, 2), dtype=np.int64)
        for m in cm:
            sl = r2[m] - c * RPC
            msg_row[m] = slot_tile_off[sl] * P + fill[sl]
            fill[sl] += 1
            p = peer[m]
            hh = half[m]
            pos = rfill[p, sub[m]]
            rfill[p, sub[m]] += 1
            base = hh * SENDH + p * SEG
            if pos < R1M:
                send_slot[m] = base + ksub[m] * R1M + pos
            else:
                op_ = ofill[p, hh]
                assert op_ < OVER
                ofill[p, hh] += 1
                of_pos[m] = op_
                send_slot[m] = base + HSUB * R1M + op_
    TRASH = NMSG                      # worst case: every GEMM row is a pad
    SENDALL = 2 * SENDH + TRASH

    cores = []
    for c in range(NCORES):
        cm = np.nonzero(owner == c)[0]
        idxA = np.zeros(NMSG, dtype=np.int64)
        idxA[msg_row[cm]] = src[cm]
        # layer-1 source positions in the split-AllGather h_full layout:
        # token t lives at (t//BLK)*(BLK//2) + (t%BLK)  [+ N//2 if in the
        # upper half of its block]
        t = idxA
        lower = (t % BLK) < (BLK // 2)
        idxA2 = np.where(
            lower,
            (t // BLK) * (BLK // 2) + (t % BLK),
            N // 2 + (t // BLK) * (BLK // 2) + (t % BLK) - (BLK // 2),
        )

        # scatter slots in GEMM-row order; pads go to unique trash rows
        idxS = np.zeros(NMSG, dtype=np.int64)
        idxS[:] = 2 * SENDH + np.arange(NMSG)     # default: trash
        idxS[msg_row[cm]] = send_slot[cm]

        # one-hot matrices against the strided recv-load layout.
        # main load for sub k: rows rr = s*R1M + pos -> (partition rr//J2M,
        # chunk rr%J2M).  overflow load (per half, shared by its subs):
        # rows rr2 = s*OVER + ofpos -> (partition rr2//J2O, chunk rr2%J2O).
        S = np.zeros((NSUB, J2M + J2O, P, P), dtype=np.float32)
        dm = np.nonzero(peer == c)[0]
        for m in dm:
            k = sub[m]
            d = (dest[m] - c * BLK) % P
            if of_pos[m] < 0:
                pos = send_slot[m] - half[m] * SENDH - c * SEG - ksub[m] * R1M
                rr = owner[m] * R1M + pos
                S[k, rr % J2M, rr // J2M, d] = 1.0
            else:
                rr2 = owner[m] * OVER + of_pos[m]
                S[k, J2M + rr2 % J2O, rr2 // J2O, d] = 1.0

        CT = np.zeros((1 + TWO_R, BLK), dtype=np.float32)
        CT[0, :] = 1.0
        for m in dm:
            CT[1 + r2[m], dest[m] - c * BLK] += 1.0

        cores.append(
            dict(
                idxA=_pack_idx16(idxA),
                idxA2=_pack_idx16(idxA2),
                idxS=_pack_idx16(idxS),
                S=S.reshape(NSUB * (J2M + J2O) * P, P).astype(BF16),
                CT=CT.astype(BF16),
            )
        )

    return dict(
        MT=MT, tile_slot=tile_slot, NMSG=NMSG, R1M=R1M, OVER=OVER, SEG=SEG,
        SENDH=SENDH, SENDALL=SENDALL, J2M=J2M, J2O=J2O, NCHUNK=NCHUNK,
        cores=cores,
    )


def _build(MT, tile_slot, NMSG, R1M, OVER, SEG, SENDH, SENDALL, J2M, J2O, NCHUNK):
    nc = bacc.Bacc(
        "TRN2",
        target_bir_lowering=False,
        debug=False,
        enable_asserts=True,
        num_devices=NCORES,
    )
    dt = mybir.dt

    x0 = nc.dram_tensor("x0", [N, D], dt.bfloat16, kind="ExternalInput")
    x_own = nc.dram_tensor("x_own", [BLK, D], dt.bfloat16, kind="ExternalInput")
    wrel = nc.dram_tensor("wrel", [L, RPC, D, D], dt.bfloat16, kind="ExternalInput")
    wselfT = nc.dram_tensor("wselfT", [L, D, D], dt.bfloat16, kind="ExternalInput")
    bias = nc.dram_tensor("bias", [L, 1 + TWO_R, D], dt.bfloat16, kind="ExternalInput")
    ct = nc.dram_tensor("ct", [1 + TWO_R, BLK], dt.bfloat16, kind="ExternalInput")
    wffT = nc.dram_tensor("wffT", [D, OUT], dt.bfloat16, kind="ExternalInput")
    bff = nc.dram_tensor("bff", [1, OUT], dt.bfloat16, kind="ExternalInput")
    idxA = nc.dram_tensor("idxA", [P, NMSG // 16], dt.int16, kind="ExternalInput")
    idxA2 = nc.dram_tensor("idxA2", [P, NMSG // 16], dt.int16, kind="ExternalInput")
    idxS = nc.dram_tensor("idxS", [P, NMSG // 16], dt.int16, kind="ExternalInput")
    idxQ = nc.dram_tensor("idxQ", [P, BLK // 16], dt.int16, kind="ExternalInput")
    s_in = nc.dram_tensor("s", [NCHUNK * P, P], dt.bfloat16, kind="ExternalInput")
    y = nc.dram_tensor("y", [BLK, OUT], dt.float32, kind="ExternalOutput")

    h_own = nc.dram_tensor("h_own", [BLK, D], dt.bfloat16)
    h_full = nc.dram_tensor("h_full", [N, D], dt.bfloat16, addr_space="Shared")
    h2_own = nc.dram_tensor("h2_own", [BLK, D], dt.bfloat16)
    send_all = nc.dram_tensor("send_all", [SENDALL, D], dt.bfloat16)
    recv_bufs = [
        nc.dram_tensor(f"recv{hh}", [SENDH, D], dt.bfloat16) for hh in range(2)
    ]

    Relu = mybir.ActivationFunctionType.Relu

    with tile.TileContext(nc) as tc:
        with (
            tc.tile_pool(name="const", bufs=1) as const,
            tc.tile_pool(name="xtc", bufs=2) as xtcp,
            tc.tile_pool(name="xself", bufs=1) as xsp,
            tc.tile_pool(name="mso", bufs=2) as msop,
            tc.tile_pool(name="msgb", bufs=4) as msgbp,
            tc.tile_pool(name="selfb", bufs=8) as selfbp,
            tc.tile_pool(name="h", bufs=3) as hp,
            tc.tile_pool(name="psum_m", bufs=4, space="PSUM") as psum_m,
            tc.tile_pool(name="psum_o", bufs=2, space="PSUM") as psum_o,
            tc.tile_pool(name="psum_y", bufs=2, space="PSUM") as psum_y,
        ):
            # ---- constants; startup-critical loads first ----
            xself0 = xsp.tile([P, KC, BLK], dt.bfloat16, tag="xself")
            nc.sync.dma_start_transpose(xself0[:], x_own.ap())

            idxA_sb = const.tile([P, NMSG // 16], dt.int16)
            nc.sync.dma_start(idxA_sb[:], idxA.ap())
            idxA2_sb = const.tile([P, NMSG // 16], dt.int16)
            nc.scalar.dma_start(idxA2_sb[:], idxA2.ap())

            wselfT_sb = const.tile([P, L, KC, D], dt.bfloat16)
            nc.sync.dma_start(
                wselfT_sb[:], wselfT.ap().rearrange("l (c p) n -> p l c n", p=P)
            )
            ct_sb = const.tile([1 + TWO_R, BLK], dt.bfloat16)
            nc.scalar.dma_start(ct_sb[:], ct.ap())
            bias_sb = const.tile([1 + TWO_R, L, D], dt.bfloat16)
            nc.scalar.dma_start(bias_sb[:], bias.ap().rearrange("l b d -> b l d"))

            wrel_sb = [[None] * RPC for _ in range(L)]
            for ll in range(L):
                for ss in range(RPC):
                    wt = const.tile([P, KC, D], dt.bfloat16, tag=f"wrel{ll}_{ss}")
                    eng = nc.sync if (ll * RPC + ss) % 2 == 0 else nc.scalar
                    eng.dma_start(
                        wt[:], wrel.ap()[ll, ss].rearrange("(c p) n -> p c n", p=P)
                    )
                    wrel_sb[ll][ss] = wt

            idxS_sb = const.tile([P, NMSG // 16], dt.int16)
            idxQ_sb = const.tile([P, BLK // 16], dt.int16)
            nc.scalar.dma_start(idxS_sb[:], idxS.ap())
            nc.scalar.dma_start(idxQ_sb[:], idxQ.ap())
            s_sb = const.tile([P, NCHUNK, P], dt.bfloat16)
            nc.scalar.dma_start(s_sb[:], s_in.ap().rearrange("(c p) n -> p c n", p=P))
            wffT_sb = const.tile([P, KC, OUT], dt.bfloat16)
            nc.scalar.dma_start(wffT_sb[:], wffT.ap().rearrange("(c p) n -> p c n", p=P))
            bff_sb = const.tile([1, OUT], dt.bfloat16)
            nc.scalar.dma_start(bff_sb[:], bff.ap())
            ones_sb = const.tile([1, P], dt.bfloat16)
            nc.vector.memset(ones_sb[:], 1.0)
            zero_sb = const.tile([P, 8, D], dt.bfloat16)
            nc.vector.memset(zero_sb[:], 0.0)

            n_ch = (NMSG + CH_IDX - 1) // CH_IDX
            tiles_per_ch = CH_IDX // P
            n_grp = (MT + SC_TILES - 1) // SC_TILES

            def selfb_compute(layer, xs_of_k):
                tiles = []
                for k in range(NSUB):
                    xt, off = xs_of_k(k)
                    pm = psum_m.tile([P, D], dt.float32, space="PSUM", tag="pmsg")
                    for kc in range(KC):
                        nc.tensor.matmul(
                            out=pm[:],
                            lhsT=xt[:, kc, off : off + P],
                            rhs=wselfT_sb[:, layer, kc, :],
                            start=(kc == 0),
                            stop=False,
                        )
                    nc.tensor.matmul(
                        out=pm[:],
                        lhsT=ct_sb[:, k * P : (k + 1) * P],
                        rhs=bias_sb[:, layer, :],
                        start=False,
                        stop=True,
                    )
                    sb = selfbp.tile([P, D], dt.float32, tag="selfb")
                    nc.vector.tensor_copy(sb[:], pm[:])
                    tiles.append(sb)
                return tiles

            def zero_wire():
                # zero the wire region [0 : 2*SENDH) of the send image
                rows = 2 * SENDH
                zrows = P * 8
                for lo in range(0, rows, zrows):
                    hi = min(lo + zrows, rows)
                    nc.sync.dma_start(
                        send_all.ap()[lo:hi, :],
                        zero_sb[:, : (hi - lo) // P, :],
                    )

            def msg_phase(layer, src, idx_sb):
                grp_tile = None
                for ci in range(n_ch):
                    lo = ci * CH_IDX
                    hi = min(lo + CH_IDX, NMSG)
                    xc = xtcp.tile([P, KC, hi - lo], dt.bfloat16, tag="xTc")
                    nc.gpsimd.dma_gather(
                        out_ap=xc[:],
                        in_ap=src.ap(),
                        idxs_ap=idx_sb[:, lo // 16 : hi // 16],
                        num_idxs=hi - lo,
                        num_idxs_reg=hi - lo,
                        elem_size=D,
                        transpose=True,
                    )
                    for ti in range((hi - lo) // P):
                        mt = ci * tiles_per_ch + ti
                        g, gslot = divmod(mt, SC_TILES)
                        if gslot == 0:
                            grp_tile = msop.tile(
                                [P, SC_TILES, D], dt.bfloat16, tag="mso"
                            )
                        ss = tile_slot[mt]
                        pm = psum_m.tile([P, D], dt.float32, space="PSUM", tag="pmsg")
                        for kc in range(KC):
                            nc.tensor.matmul(
                                out=pm[:],
                                lhsT=xc[:, kc, ti * P : (ti + 1) * P],
                                rhs=wrel_sb[layer][ss][:, kc, :],
                                start=(kc == 0),
                                stop=(kc == KC - 1),
                            )
                        nc.vector.tensor_copy(grp_tile[:, gslot, :], pm[:])
                        if gslot == SC_TILES - 1 or mt == MT - 1:
                            nidx = (gslot + 1) * P
                            nc.gpsimd.dma_scatter_add(
                                send_all.ap(),
                                grp_tile[:, : gslot + 1, :],
                                idxS_sb[:, g * SC_TILES * P // 16 :
                                        (g * SC_TILES + gslot + 1) * P // 16],
                                nidx,
                                nidx,
                                D,
                            )

            def a2a(hh):
                nc.gpsimd.collective_compute(
                    "AllToAll",
                    mybir.AluOpType.bypass,
                    replica_groups=[list(range(NCORES))],
                    ins=[send_all.ap()[hh * SENDH : (hh + 1) * SENDH, :]],
                    outs=[recv_bufs[hh].ap()],
                )

            def accum_half(layer, hh, selfb, h_out, ff=False, h2T_tiles=None):
                seg = recv_bufs[hh].ap().rearrange("(s g) d -> s g d", s=NCORES)
                # eager loads on the scalar queue so they never sit behind
                # the h-writes of earlier sub-blocks
                ov = msgbp.tile([P, J2O, D], dt.bfloat16, tag="msgO")
                nc.scalar.dma_start(
                    ov[:], seg[:, HSUB * R1M : HSUB * R1M + OVER, :]
                )
                mbs = []
                for kl in range(HSUB):
                    mb = msgbp.tile([P, J2M, D], dt.bfloat16, tag="msgB")
                    nc.scalar.dma_start(
                        mb[:], seg[:, kl * R1M : (kl + 1) * R1M, :]
                    )
                    mbs.append(mb)
                JT = J2M + J2O
                for kl in range(HSUB):
                    k = hh * HSUB + kl
                    mb = mbs[kl]
                    po = psum_o.tile([P, D], dt.float32, space="PSUM", tag="pout")
                    for j in range(J2M):
                        nc.tensor.matmul(
                            out=po[:],
                            lhsT=s_sb[:, k * JT + j, :],
                            rhs=mb[:, j, :],
                            start=(j == 0),
                            stop=False,
                        )
                    for j in range(J2O):
                        nc.tensor.matmul(
                            out=po[:],
                            lhsT=s_sb[:, k * JT + J2M + j, :],
                            rhs=ov[:, j, :],
                            start=False,
                            stop=(j == J2O - 1),
                        )
                    nc.vector.tensor_add(out=po[:], in0=po[:], in1=selfb[k][:])
                    hsb = hp.tile([P, D], dt.bfloat16, tag="hsb")
                    nc.vector.tensor_scalar(
                        hsb[:], po[:], 0.0, None, mybir.AluOpType.max
                    )
                    nc.sync.dma_start(h_out.ap()[k * P : (k + 1) * P, :], hsb[:])
                    if ff:
                        # final layer: fold the FF GEMM for this sub-block in
                        # right away (transposing gather of the 128 rows just
                        # written), overlapping FF with the accumulation.
                        xt = xsp.tile([P, KC, P], dt.bfloat16, tag="h2T")
                        nc.gpsimd.dma_gather(
                            out_ap=xt[:],
                            in_ap=h_out.ap(),
                            idxs_ap=idxQ_sb[:, k * P // 16 : (k + 1) * P // 16],
                            num_idxs=P,
                            num_idxs_reg=P,
                            elem_size=D,
                            transpose=True,
                        )
                        py_ = psum_y.tile([P, OUT], dt.float32, space="PSUM", tag="py")
                        for kc in range(KC):
                            nc.tensor.matmul(
                                out=py_[:],
                                lhsT=xt[:, kc, :],
                                rhs=wffT_sb[:, kc, :],
                                start=(kc == 0),
                                stop=False,
                            )
                        nc.tensor.matmul(
                            out=py_[:], lhsT=ones_sb[:], rhs=bff_sb[:],
                            start=False, stop=True,
                        )
                        ysb = hp.tile([P, OUT], dt.float32, tag="ysb")
                        nc.vector.tensor_copy(ysb[:], py_[:])
                        nc.sync.dma_start(y.ap()[k * P : (k + 1) * P, :], ysb[:])

            def run_layer(layer, src, idx_sb, selfb, h_out, ff=False,
                          ag_split=False):
                zero_wire()
                msg_phase(layer, src, idx_sb)
                a2a(0)
                a2a(1)
                accum_half(layer, 0, selfb, h_out, ff=ff)
                if ag_split:
                    # AllGather of the first token-half right away; overlaps
                    # the second half's accumulation.
                    nc.gpsimd.collective_compute(
                        "AllGather",
                        mybir.AluOpType.bypass,
                        replica_groups=[list(range(NCORES))],
                        ins=[h_own.ap()[: BLK // 2, :]],
                        outs=[h_full.ap()[: N // 2, :]],
                    )
                accum_half(layer, 1, selfb, h_out, ff=ff)
                if ag_split:
                    nc.gpsimd.collective_compute(
                        "AllGather",
                        mybir.AluOpType.bypass,
                        replica_groups=[list(range(NCORES))],
                        ins=[h_own.ap()[BLK // 2 :, :]],
                        outs=[h_full.ap()[N // 2 :, :]],
                    )

            # tiny warm-up collectives: pay the ncfw/ring cold-start cost
            # during the (DMA-bound) startup instead of on layer 0's A2A.
            warm_in = nc.dram_tensor("warm_in", [16, 64], dt.bfloat16)
            warm_out = nc.dram_tensor("warm_out", [16, 64], dt.bfloat16)
            warm_ag = nc.dram_tensor(
                "warm_ag", [128, 64], dt.bfloat16, addr_space="Shared"
            )
            nc.sync.dma_start(warm_in.ap(), zero_sb[:16, 0, :64])
            nc.gpsimd.collective_compute(
                "AllToAll",
                mybir.AluOpType.bypass,
                replica_groups=[list(range(NCORES))],
                ins=[warm_in.ap()],
                outs=[warm_out.ap()],
            )
            nc.gpsimd.collective_compute(
                "AllGather",
                mybir.AluOpType.bypass,
                replica_groups=[list(range(NCORES))],
                ins=[warm_in.ap()],
                outs=[warm_ag.ap()],
            )

            # ================= layer 0 =================
            selfb0 = selfb_compute(0, lambda k: (xself0, k * P))
            run_layer(0, x0, idxA_sb, selfb0, h_own, ag_split=True)

            # layer-1 self work (fills the second AllGather's dead time)
            xq = []
            nxq = (BLK + CH_IDX - 1) // CH_IDX
            for ci in range(nxq):
                lo = ci * CH_IDX
                hi = min(lo + CH_IDX, BLK)
                xc = xsp.tile([P, KC, hi - lo], dt.bfloat16, tag=f"xq{ci}")
                nc.gpsimd.dma_gather(
                    out_ap=xc[:],
                    in_ap=h_own.ap(),
                    idxs_ap=idxQ_sb[:, lo // 16 : hi // 16],
                    num_idxs=hi - lo,
                    num_idxs_reg=hi - lo,
                    elem_size=D,
                    transpose=True,
                )
                xq.append(xc)
            selfb1 = selfb_compute(
                1, lambda k: (xq[(k * P) // CH_IDX], k * P - ((k * P) // CH_IDX) * CH_IDX)
            )

            # ================= layer 1 (with fused FF) =================
            run_layer(1, h_full, idxA2_sb, selfb1, h2_own, ff=True)

    nc.compile()
    return nc


def _in_maps(plan, x, W_self, b_self, W_rel, b_rel, W_ff, b_ff):
    x0 = x.astype(BF16)
    wselfT = np.ascontiguousarray(W_self.transpose(0, 2, 1)).astype(BF16)
    bias = np.concatenate([b_self[:, None, :], b_rel], axis=1).astype(BF16)
    wffT = np.ascontiguousarray(W_ff.T).astype(BF16)
    bffr = b_ff.reshape(1, OUT).astype(BF16)
    wrelT_all = np.ascontiguousarray(W_rel.transpose(0, 1, 3, 2)).astype(BF16)
    idxQ = _pack_idx16(np.arange(BLK))

    in_maps = []
    for c in range(NCORES):
        t = plan["cores"][c]
        in_maps.append(
            {
                "x0": x0,
                "x_own": np.ascontiguousarray(x0[c * BLK : (c + 1) * BLK]),
                "wrel": np.ascontiguousarray(wrelT_all[:, c * RPC : (c + 1) * RPC]),
                "wselfT": wselfT,
                "bias": bias,
                "ct": t["CT"],
                "wffT": wffT,
                "bff": bffr,
                "idxA": t["idxA"],
                "idxA2": t["idxA2"],
                "idxS": t["idxS"],
                "idxQ": idxQ,
                "s": t["S"],
            }
        )
    return in_maps


def kernel(x, dep_idx, rel_idx, gov_idx, W_self, b_self, W_rel, b_rel, W_ff, b_ff):
    global LAST_EXEC_TIME_NS, LAST_RESULTS

    x = np.asarray(x)
    dep_idx = np.asarray(dep_idx)
    rel_idx = np.asarray(rel_idx)
    gov_idx = np.asarray(gov_idx)
    W_self = np.asarray(W_self)
    b_self = np.asarray(b_self)
    W_rel = np.asarray(W_rel)
    b_rel = np.asarray(b_rel)
    W_ff = np.asarray(W_ff)
    b_ff = np.asarray(b_ff)
    assert x.shape == (N, D) and W_rel.shape == (L, TWO_R, D, D)

    key = (dep_idx.tobytes(), rel_idx.tobytes(), gov_idx.tobytes())
    if key in _CACHE:
        nc, plan = _CACHE[key]
    else:
        plan = _plan(dep_idx, rel_idx, gov_idx)
        nc = _build(
            plan["MT"], plan["tile_slot"], plan["NMSG"], plan["R1M"], plan["OVER"],
            plan["SEG"], plan["SENDH"], plan["SENDALL"], plan["J2M"], plan["J2O"],
            plan["NCHUNK"],
        )
        _CACHE.clear()
        _CACHE[key] = (nc, plan)

    in_maps = _in_maps(plan, x, W_self, b_self, W_rel, b_rel, W_ff, b_ff)
    res = run_bass_kernel_spmd(nc, in_maps, list(range(NCORES)))
    LAST_EXEC_TIME_NS = res.exec_time_ns
    LAST_RESULTS = res
    out = np.concatenate([res.results[c]["y"] for c in range(NCORES)], axis=0)
    return out.astype(np.float32)


# revision 12
# speedup vs baseline: 1.0746x; 1.0123x over previous
"""Trainium2 Bass kernel for the 2-layer dependency-relation GCN (8 cores).

Math per layer l, token i:
    out[i] = relu( W_self[l] @ x[i] + b_self[l]
                   + sum_{e: dep[e]==i} (W_rel[l, rel[e]]   @ x[gov[e]] + b_rel[l, rel[e]])
                   + sum_{e: gov[e]==i} (W_rel[l, R+rel[e]] @ x[dep[e]] + b_rel[l, R+rel[e]]) )
final:  y = h @ W_ff.T + b_ff

The reference computes 40 dense [8192,512]x[512,512] GEMMs per layer; each
edge uses exactly one relation, so this kernel instead groups the 2N=16384
messages by relation and transforms only gathered source rows (13x fewer
FLOPs), relation-sharded across the 8 NeuronCores.

Pipeline per layer, per core c (owner of relations 5c..5c+4):
  1. transposing dma_gather of message sources (relation-grouped, padded to
     128-row M-tiles), chunked 256 idxs/instruction.
  2. per-tile GEMMs with SBUF-resident relation weights; PSUM -> bf16 rows
     are collected 4 tiles at a time into an SBUF staging tile.
  3. dma_scatter_add writes each staging group straight into the AllToAll
     send image (zero-filled per layer), laid out [half][peer][sub][R1 pad];
     GEMM pad rows go to a trash region past the wire window.  No message
     bounce buffer and no pack gather.
  4. two AllToAlls (sub-halves): the first half's accumulation overlaps the
     second half's wire time.
  5. per dest sub-block: ONE plain strided DMA pulls its [8 senders x R1]
     rows from the recv buffer; host-built one-hot matmuls scatter-add into
     PSUM; a DVE pass adds the precomputed self+bias tile; fused ReLU.
     Self+bias GEMMs run early (during gathers / collectives); layer 1's are
     emitted before the AllGather (via transposing gathers of h_own - XBAR
     transposes serialize against collectives) to fill its dead time.
final FF layer as before.
Numerics: bf16 matmul inputs / wire, fp32 PSUM accumulation.
"""

import numpy as np
import ml_dtypes

import concourse.bass as bass
import concourse.mybir as mybir
import concourse.tile as tile
from concourse import bacc
from concourse.bass_utils import run_bass_kernel_spmd

N = 8192
D = 512
R = 20
TWO_R = 2 * R
L = 2
OUT = 256
P = 128
NCORES = 8
RPC = TWO_R // NCORES    # 5 relations per core
BLK = N // NCORES        # 1024 tokens per core
NSUB = BLK // P          # 8 sub-blocks of 128 tokens
HSUB = NSUB // 2
KC = D // P              # 4 contraction chunks
CH_IDX = 2 * P           # idxs per transposing-gather chunk
SC_TILES = 4             # GEMM tiles per scatter-add group

BF16 = ml_dtypes.bfloat16

LAST_EXEC_TIME_NS = None
LAST_RESULTS = None

_CACHE = {}


def _pack_idx16(idx: np.ndarray) -> np.ndarray:
    Ln = len(idx)
    assert Ln % 16 == 0
    base = idx.astype(np.int16).reshape(Ln // 16, 16).T
    return np.tile(base, (8, 1)).copy()


def _plan(dep_idx: np.ndarray, rel_idx: np.ndarray, gov_idx: np.ndarray):
    dep = dep_idx.astype(np.int64)
    gov = gov_idx.astype(np.int64)
    rel = rel_idx.astype(np.int64)

    dest = np.concatenate([dep, gov])
    src = np.concatenate([gov, dep])
    r2 = np.concatenate([rel, rel + R])

    owner = r2 // RPC
    peer = dest // BLK
    sub = (dest % BLK) // P
    half = sub // HSUB
    ksub = sub % HSUB                 # sub index within the half

    # GEMM tiling: tiles per relation-slot, max over cores
    tps = []
    for s in range(RPC):
        mx = 1
        for c in range(NCORES):
            n = int((r2 == c * RPC + s).sum())
            mx = max(mx, int(np.ceil(n / P)))
        tps.append(mx)
    MT = sum(tps)
    tile_slot = []
    slot_tile_off = []
    off = 0
    for s in range(RPC):
        slot_tile_off.append(off)
        tile_slot.extend([s] * tps[s])
        off += tps[s]
    NMSG = MT * P

    # wire layout per half-segment for peer p:
    #   [ksub 0..3][R1M rows each] ++ [overflow block: OVER rows]
    # Cells larger than R1M spill into the per-(owner,peer,half) overflow
    # block; the receiver one-hots the shared overflow block per sub-block.
    cnt = np.zeros((NCORES, NCORES, NSUB), dtype=np.int64)
    for m in range(2 * N):
        cnt[owner[m], peer[m], sub[m]] += 1

    def over_for(r1m):
        ov = 0
        for c in range(NCORES):
            for p in range(NCORES):
                for hh in range(2):
                    tot = sum(
                        max(0, int(cnt[c, p, hh * HSUB + kl]) - r1m)
                        for kl in range(HSUB)
                    )
                    ov = max(ov, tot)
        return int(np.ceil(ov / 16) * 16) if ov else 0

    best = None
    for r1m in (16, 32, 48, 64):
        ov = over_for(r1m)
        seg = HSUB * r1m + ov
        # cost: wire bytes (seg) primary; one-hot matmul chunks secondary
        # (overflow chunks are one-hot'd once per sub-block -> 4x weight)
        chunks = (NCORES * r1m) // P + (NCORES * ov) // P
        key = (seg, chunks)
        if best is None or key < best[0]:
            best = (key, r1m, ov, seg)
    _, R1M, OVER, SEG = best
    if OVER == 0:
        OVER = 16
        SEG = HSUB * R1M + OVER
    SENDH = NCORES * SEG              # wire rows per half per rank
    J2M = NCORES * R1M // P
    J2O = NCORES * OVER // P
    assert (NCORES * R1M) % P == 0 and (NCORES * OVER) % P == 0
    NCHUNK = NSUB * (J2M + J2O)

    # per-message assignment
    msg_row = np.zeros(2 * N, dtype=np.int64)     # GEMM-output row (owner)
    send_slot = np.zeros(2 * N, dtype=np.int64)   # row in send image (owner)
    of_pos = np.zeros(2 * N, dtype=np.int64) - 1  # pos in overflow block
    for c in range(NCORES):
        cm = np.nonzero(owner == c)[0]
        fill = np.zeros(RPC, dtype=np.int64)
        rfill = np.zeros((NCORES, NSUB), dtype=np.int64)
        ofill = np.zeros((NCORES, 2), dtype=np.int64)
        for m in cm:
            sl = r2[m] - c * RPC
            msg_row[m] = slot_tile_off[sl] * P + fill[sl]
            fill[sl] += 1
            p = peer[m]
            hh = half[m]
            pos = rfill[p, sub[m]]
            rfill[p, sub[m]] += 1
            base = hh * SENDH + p * SEG
            if pos < R1M:
                send_slot[m] = base + ksub[m] * R1M + pos
            else:
                op_ = ofill[p, hh]
                assert op_ < OVER
                ofill[p, hh] += 1
                of_pos[m] = op_
                send_slot[m] = base + HSUB * R1M + op_
    TRASH = NMSG                      # worst case: every GEMM row is a pad
    SENDALL = 2 * SENDH + TRASH

    cores = []
    for c in range(NCORES):
        cm = np.nonzero(owner == c)[0]
        idxA = np.zeros(NMSG, dtype=np.int64)
        idxA[msg_row[cm]] = src[cm]
        # layer-1 source positions in the split-AllGather h_full layout:
        # token t lives at (t//BLK)*(BLK//2) + (t%BLK)  [+ N//2 if in the
        # upper half of its block]
        t = idxA
        lower = (t % BLK) < (BLK // 2)
        idxA2 = np.where(
            lower,
            (t // BLK) * (BLK // 2) + (t % BLK),
            N // 2 + (t // BLK) * (BLK // 2) + (t % BLK) - (BLK // 2),
        )

        # scatter slots in GEMM-row order; pads go to unique trash rows
        idxS = np.zeros(NMSG, dtype=np.int64)
        idxS[:] = 2 * SENDH + np.arange(NMSG)     # default: trash
        idxS[msg_row[cm]] = send_slot[cm]

        # one-hot matrices against the strided recv-load layout.
        # main load for sub k: rows rr = s*R1M + pos -> (partition rr//J2M,
        # chunk rr%J2M).  overflow load (per half, shared by its subs):
        # rows rr2 = s*OVER + ofpos -> (partition rr2//J2O, chunk rr2%J2O).
        S = np.zeros((NSUB, J2M + J2O, P, P), dtype=np.float32)
        dm = np.nonzero(peer == c)[0]
        for m in dm:
            k = sub[m]
            d = (dest[m] - c * BLK) % P
            if of_pos[m] < 0:
                pos = send_slot[m] - half[m] * SENDH - c * SEG - ksub[m] * R1M
                rr = owner[m] * R1M + pos
                S[k, rr % J2M, rr // J2M, d] = 1.0
            else:
                rr2 = owner[m] * OVER + of_pos[m]
                S[k, J2M + rr2 % J2O, rr2 // J2O, d] = 1.0

        CT = np.zeros((1 + TWO_R, BLK), dtype=np.float32)
        CT[0, :] = 1.0
        for m in dm:
            CT[1 + r2[m], dest[m] - c * BLK] += 1.0

        cores.append(
            dict(
                idxA=_pack_idx16(idxA),
                idxA2=_pack_idx16(idxA2),
                idxS=_pack_idx16(idxS),
                S=S.reshape(NSUB * (J2M + J2O) * P, P).astype(BF16),
                CT=CT.astype(BF16),
            )
        )

    return dict(
        MT=MT, tile_slot=tile_slot, NMSG=NMSG, R1M=R1M, OVER=OVER, SEG=SEG,
        SENDH=SENDH, SENDALL=SENDALL, J2M=J2M, J2O=J2O, NCHUNK=NCHUNK,
        cores=cores,
    )


def _build(MT, tile_slot, NMSG, R1M, OVER, SEG, SENDH, SENDALL, J2M, J2O, NCHUNK):
    nc = bacc.Bacc(
        "TRN2",
        target_bir_lowering=False,
        debug=False,
        enable_asserts=True,
        num_devices=NCORES,
    )
    dt = mybir.dt

    x0 = nc.dram_tensor("x0", [N, D], dt.bfloat16, kind="ExternalInput")
    x_own = nc.dram_tensor("x_own", [BLK, D], dt.bfloat16, kind="ExternalInput")
    wrel = nc.dram_tensor("wrel", [L, RPC, D, D], dt.bfloat16, kind="ExternalInput")
    wselfT = nc.dram_tensor("wselfT", [L, D, D], dt.bfloat16, kind="ExternalInput")
    bias = nc.dram_tensor("bias", [L, 1 + TWO_R, D], dt.bfloat16, kind="ExternalInput")
    ct = nc.dram_tensor("ct", [1 + TWO_R, BLK], dt.bfloat16, kind="ExternalInput")
    wffT = nc.dram_tensor("wffT", [D, OUT], dt.bfloat16, kind="ExternalInput")
    bff = nc.dram_tensor("bff", [1, OUT], dt.bfloat16, kind="ExternalInput")
    idxA = nc.dram_tensor("idxA", [P, NMSG // 16], dt.int16, kind="ExternalInput")
    idxA2 = nc.dram_tensor("idxA2", [P, NMSG // 16], dt.int16, kind="ExternalInput")
    idxS = nc.dram_tensor("idxS", [P, NMSG // 16], dt.int16, kind="ExternalInput")
    idxQ = nc.dram_tensor("idxQ", [P, BLK // 16], dt.int16, kind="ExternalInput")
    s_in = nc.dram_tensor("s", [NCHUNK * P, P], dt.bfloat16, kind="ExternalInput")
    y = nc.dram_tensor("y", [BLK, OUT], dt.float32, kind="ExternalOutput")

    h_own = nc.dram_tensor("h_own", [BLK, D], dt.bfloat16)
    h_full = nc.dram_tensor("h_full", [N, D], dt.bfloat16, addr_space="Shared")
    h2_own = nc.dram_tensor("h2_own", [BLK, D], dt.bfloat16)
    send_all = nc.dram_tensor("send_all", [SENDALL, D], dt.bfloat16)
    recv_bufs = [
        nc.dram_tensor(f"recv{hh}", [SENDH, D], dt.bfloat16) for hh in range(2)
    ]

    Relu = mybir.ActivationFunctionType.Relu

    with tile.TileContext(nc) as tc:
        with (
            tc.tile_pool(name="const", bufs=1) as const,
            tc.tile_pool(name="xtc", bufs=2) as xtcp,
            tc.tile_pool(name="xself", bufs=1) as xsp,
            tc.tile_pool(name="mso", bufs=2) as msop,
            tc.tile_pool(name="msgb", bufs=8) as msgbp,
            tc.tile_pool(name="selfb", bufs=8) as selfbp,
            tc.tile_pool(name="h", bufs=3) as hp,
            tc.tile_pool(name="psum_m", bufs=4, space="PSUM") as psum_m,
            tc.tile_pool(name="psum_o", bufs=2, space="PSUM") as psum_o,
            tc.tile_pool(name="psum_y", bufs=2, space="PSUM") as psum_y,
        ):
            # ---- constants; startup-critical loads first ----
            xself0 = xsp.tile([P, KC, BLK], dt.bfloat16, tag="xself")
            nc.sync.dma_start_transpose(xself0[:], x_own.ap())

            idxA_sb = const.tile([P, NMSG // 16], dt.int16)
            nc.sync.dma_start(idxA_sb[:], idxA.ap())
            idxA2_sb = const.tile([P, NMSG // 16], dt.int16)
            nc.scalar.dma_start(idxA2_sb[:], idxA2.ap())

            wselfT_sb = const.tile([P, L, KC, D], dt.bfloat16)
            nc.sync.dma_start(
                wselfT_sb[:], wselfT.ap().rearrange("l (c p) n -> p l c n", p=P)
            )
            ct_sb = const.tile([1 + TWO_R, BLK], dt.bfloat16)
            nc.scalar.dma_start(ct_sb[:], ct.ap())
            bias_sb = const.tile([1 + TWO_R, L, D], dt.bfloat16)
            nc.scalar.dma_start(bias_sb[:], bias.ap().rearrange("l b d -> b l d"))

            wrel_sb = [[None] * RPC for _ in range(L)]
            for ll in range(L):
                for ss in range(RPC):
                    wt = const.tile([P, KC, D], dt.bfloat16, tag=f"wrel{ll}_{ss}")
                    eng = nc.sync if (ll * RPC + ss) % 2 == 0 else nc.scalar
                    eng.dma_start(
                        wt[:], wrel.ap()[ll, ss].rearrange("(c p) n -> p c n", p=P)
                    )
                    wrel_sb[ll][ss] = wt

            idxS_sb = const.tile([P, NMSG // 16], dt.int16)
            idxQ_sb = const.tile([P, BLK // 16], dt.int16)
            nc.scalar.dma_start(idxS_sb[:], idxS.ap())
            nc.scalar.dma_start(idxQ_sb[:], idxQ.ap())
            s_sb = const.tile([P, NCHUNK, P], dt.bfloat16)
            nc.scalar.dma_start(s_sb[:], s_in.ap().rearrange("(c p) n -> p c n", p=P))
            wffT_sb = const.tile([P, KC, OUT], dt.bfloat16)
            nc.scalar.dma_start(wffT_sb[:], wffT.ap().rearrange("(c p) n -> p c n", p=P))
            bff_sb = const.tile([1, OUT], dt.bfloat16)
            nc.scalar.dma_start(bff_sb[:], bff.ap())
            ones_sb = const.tile([1, P], dt.bfloat16)
            nc.vector.memset(ones_sb[:], 1.0)
            zero_sb = const.tile([P, 8, D], dt.bfloat16)
            nc.vector.memset(zero_sb[:], 0.0)

            n_ch = (NMSG + CH_IDX - 1) // CH_IDX
            tiles_per_ch = CH_IDX // P
            n_grp = (MT + SC_TILES - 1) // SC_TILES

            def selfb_compute(layer, xs_of_k):
                tiles = []
                for k in range(NSUB):
                    xt, off = xs_of_k(k)
                    pm = psum_m.tile([P, D], dt.float32, space="PSUM", tag="pmsg")
                    for kc in range(KC):
                        nc.tensor.matmul(
                            out=pm[:],
                            lhsT=xt[:, kc, off : off + P],
                            rhs=wselfT_sb[:, layer, kc, :],
                            start=(kc == 0),
                            stop=False,
                        )
                    nc.tensor.matmul(
                        out=pm[:],
                        lhsT=ct_sb[:, k * P : (k + 1) * P],
                        rhs=bias_sb[:, layer, :],
                        start=False,
                        stop=True,
                    )
                    sb = selfbp.tile([P, D], dt.float32, tag="selfb")
                    nc.vector.tensor_copy(sb[:], pm[:])
                    tiles.append(sb)
                return tiles

            def zero_wire():
                # zero the wire region [0 : 2*SENDH) of the send image
                rows = 2 * SENDH
                zrows = P * 8
                for lo in range(0, rows, zrows):
                    hi = min(lo + zrows, rows)
                    nc.sync.dma_start(
                        send_all.ap()[lo:hi, :],
                        zero_sb[:, : (hi - lo) // P, :],
                    )

            def msg_phase(layer, src, idx_sb):
                grp_tile = None
                for ci in range(n_ch):
                    lo = ci * CH_IDX
                    hi = min(lo + CH_IDX, NMSG)
                    xc = xtcp.tile([P, KC, hi - lo], dt.bfloat16, tag="xTc")
                    nc.gpsimd.dma_gather(
                        out_ap=xc[:],
                        in_ap=src.ap(),
                        idxs_ap=idx_sb[:, lo // 16 : hi // 16],
                        num_idxs=hi - lo,
                        num_idxs_reg=hi - lo,
                        elem_size=D,
                        transpose=True,
                    )
                    for ti in range((hi - lo) // P):
                        mt = ci * tiles_per_ch + ti
                        g, gslot = divmod(mt, SC_TILES)
                        if gslot == 0:
                            grp_tile = msop.tile(
                                [P, SC_TILES, D], dt.bfloat16, tag="mso"
                            )
                        ss = tile_slot[mt]
                        pm = psum_m.tile([P, D], dt.float32, space="PSUM", tag="pmsg")
                        for kc in range(KC):
                            nc.tensor.matmul(
                                out=pm[:],
                                lhsT=xc[:, kc, ti * P : (ti + 1) * P],
                                rhs=wrel_sb[layer][ss][:, kc, :],
                                start=(kc == 0),
                                stop=(kc == KC - 1),
                            )
                        nc.vector.tensor_copy(grp_tile[:, gslot, :], pm[:])
                        if gslot == SC_TILES - 1 or mt == MT - 1:
                            nidx = (gslot + 1) * P
                            nc.gpsimd.dma_scatter_add(
                                send_all.ap(),
                                grp_tile[:, : gslot + 1, :],
                                idxS_sb[:, g * SC_TILES * P // 16 :
                                        (g * SC_TILES + gslot + 1) * P // 16],
                                nidx,
                                nidx,
                                D,
                            )

            def a2a(hh):
                nc.gpsimd.collective_compute(
                    "AllToAll",
                    mybir.AluOpType.bypass,
                    replica_groups=[list(range(NCORES))],
                    ins=[send_all.ap()[hh * SENDH : (hh + 1) * SENDH, :]],
                    outs=[recv_bufs[hh].ap()],
                )

            def accum_half(layer, hh, selfb, h_out, ff=False, h2T_tiles=None):
                seg = recv_bufs[hh].ap().rearrange("(s g) d -> s g d", s=NCORES)
                # eager loads on the scalar queue so they never sit behind
                # the h-writes of earlier sub-blocks
                ov = msgbp.tile([P, J2O, D], dt.bfloat16, tag="msgO")
                nc.scalar.dma_start(
                    ov[:], seg[:, HSUB * R1M : HSUB * R1M + OVER, :]
                )
                mbs = []
                for kl in range(HSUB):
                    mb = msgbp.tile([P, J2M, D], dt.bfloat16, tag="msgB")
                    nc.scalar.dma_start(
                        mb[:], seg[:, kl * R1M : (kl + 1) * R1M, :]
                    )
                    mbs.append(mb)
                JT = J2M + J2O
                for kl in range(HSUB):
                    k = hh * HSUB + kl
                    mb = mbs[kl]
                    po = psum_o.tile([P, D], dt.float32, space="PSUM", tag="pout")
                    for j in range(J2M):
                        nc.tensor.matmul(
                            out=po[:],
                            lhsT=s_sb[:, k * JT + j, :],
                            rhs=mb[:, j, :],
                            start=(j == 0),
                            stop=False,
                        )
                    for j in range(J2O):
                        nc.tensor.matmul(
                            out=po[:],
                            lhsT=s_sb[:, k * JT + J2M + j, :],
                            rhs=ov[:, j, :],
                            start=False,
                            stop=(j == J2O - 1),
                        )
                    nc.vector.tensor_add(out=po[:], in0=po[:], in1=selfb[k][:])
                    hsb = hp.tile([P, D], dt.bfloat16, tag="hsb")
                    nc.vector.tensor_scalar(
                        hsb[:], po[:], 0.0, None, mybir.AluOpType.max
                    )
                    nc.sync.dma_start(h_out.ap()[k * P : (k + 1) * P, :], hsb[:])
                    if ff:
                        # final layer: fold the FF GEMM for this sub-block in
                        # right away (transposing gather of the 128 rows just
                        # written), overlapping FF with the accumulation.
                        xt = xsp.tile([P, KC, P], dt.bfloat16, tag="h2T")
                        nc.gpsimd.dma_gather(
                            out_ap=xt[:],
                            in_ap=h_out.ap(),
                            idxs_ap=idxQ_sb[:, k * P // 16 : (k + 1) * P // 16],
                            num_idxs=P,
                            num_idxs_reg=P,
                            elem_size=D,
                            transpose=True,
                        )
                        py_ = psum_y.tile([P, OUT], dt.float32, space="PSUM", tag="py")
                        for kc in range(KC):
                            nc.tensor.matmul(
                                out=py_[:],
                                lhsT=xt[:, kc, :],
                                rhs=wffT_sb[:, kc, :],
                                start=(kc == 0),
                                stop=False,
                            )
                        nc.tensor.matmul(
                            out=py_[:], lhsT=ones_sb[:], rhs=bff_sb[:],
                            start=False, stop=True,
                        )
                        ysb = hp.tile([P, OUT], dt.float32, tag="ysb")
                        nc.vector.tensor_copy(ysb[:], py_[:])
                        nc.sync.dma_start(y.ap()[k * P : (k + 1) * P, :], ysb[:])

            def run_layer(layer, src, idx_sb, selfb, h_out, ff=False,
                          ag_split=False):
                zero_wire()
                msg_phase(layer, src, idx_sb)
                a2a(0)
                a2a(1)
                accum_half(layer, 0, selfb, h_out, ff=ff)
                if ag_split:
                    # AllGather of the first token-half right away; overlaps
                    # the second half's accumulation.
                    nc.gpsimd.collective_compute(
                        "AllGather",
                        mybir.AluOpType.bypass,
                        replica_groups=[list(range(NCORES))],
                        ins=[h_own.ap()[: BLK // 2, :]],
                        outs=[h_full.ap()[: N // 2, :]],
                    )
                accum_half(layer, 1, selfb, h_out, ff=ff)
                if ag_split:
                    nc.gpsimd.collective_compute(
                        "AllGather",
                        mybir.AluOpType.bypass,
                        replica_groups=[list(range(NCORES))],
                        ins=[h_own.ap()[BLK // 2 :, :]],
                        outs=[h_full.ap()[N // 2 :, :]],
                    )

            # tiny warm-up collectives: pay the ncfw/ring cold-start cost
            # during the (DMA-bound) startup instead of on layer 0's A2A.
            warm_in = nc.dram_tensor("warm_in", [16, 64], dt.bfloat16)
            warm_out = nc.dram_tensor("warm_out", [16, 64], dt.bfloat16)
            warm_ag = nc.dram_tensor(
                "warm_ag", [128, 64], dt.bfloat16, addr_space="Shared"
            )
            nc.sync.dma_start(warm_in.ap(), zero_sb[:16, 0, :64])
            nc.gpsimd.collective_compute(
                "AllToAll",
                mybir.AluOpType.bypass,
                replica_groups=[list(range(NCORES))],
                ins=[warm_in.ap()],
                outs=[warm_out.ap()],
            )
            nc.gpsimd.collective_compute(
                "AllGather",
                mybir.AluOpType.bypass,
                replica_groups=[list(range(NCORES))],
                ins=[warm_in.ap()],
                outs=[warm_ag.ap()],
            )

            # ================= layer 0 =================
            selfb0 = selfb_compute(0, lambda k: (xself0, k * P))
            run_layer(0, x0, idxA_sb, selfb0, h_own, ag_split=True)

            # layer-1 self work (fills the second AllGather's dead time)
            xq = []
            nxq = (BLK + CH_IDX - 1) // CH_IDX
            for ci in range(nxq):
                lo = ci * CH_IDX
                hi = min(lo + CH_IDX, BLK)
                xc = xsp.tile([P, KC, hi - lo], dt.bfloat16, tag=f"xq{ci}")
                nc.gpsimd.dma_gather(
                    out_ap=xc[:],
                    in_ap=h_own.ap(),
                    idxs_ap=idxQ_sb[:, lo // 16 : hi // 16],
                    num_idxs=hi - lo,
                    num_idxs_reg=hi - lo,
                    elem_size=D,
                    transpose=True,
                )
                xq.append(xc)
            selfb1 = selfb_compute(
                1, lambda k: (xq[(k * P) // CH_IDX], k * P - ((k * P) // CH_IDX) * CH_IDX)
            )

            # ================= layer 1 (with fused FF) =================
            run_layer(1, h_full, idxA2_sb, selfb1, h2_own, ff=True)

    nc.compile()
    return nc


def _in_maps(plan, x, W_self, b_self, W_rel, b_rel, W_ff, b_ff):
    x0 = x.astype(BF16)
    wselfT = np.ascontiguousarray(W_self.transpose(0, 2, 1)).astype(BF16)
    bias = np.concatenate([b_self[:, None, :], b_rel], axis=1).astype(BF16)
    wffT = np.ascontiguousarray(W_ff.T).astype(BF16)
    bffr = b_ff.reshape(1, OUT).astype(BF16)
    wrelT_all = np.ascontiguousarray(W_rel.transpose(0, 1, 3, 2)).astype(BF16)
    idxQ = _pack_idx16(np.arange(BLK))

    in_maps = []
    for c in range(NCORES):
        t = plan["cores"][c]
        in_maps.append(
            {
                "x0": x0,
                "x_own": np.ascontiguousarray(x0[c * BLK : (c + 1) * BLK]),
                "wrel": np.ascontiguousarray(wrelT_all[:, c * RPC : (c + 1) * RPC]),
                "wselfT": wselfT,
                "bias": bias,
                "ct": t["CT"],
                "wffT": wffT,
                "bff": bffr,
                "idxA": t["idxA"],
                "idxA2": t["idxA2"],
                "idxS": t["idxS"],
                "idxQ": idxQ,
                "s": t["S"],
            }
        )
    return in_maps


def kernel(x, dep_idx, rel_idx, gov_idx, W_self, b_self, W_rel, b_rel, W_ff, b_ff):
    global LAST_EXEC_TIME_NS, LAST_RESULTS

    x = np.asarray(x)
    dep_idx = np.asarray(dep_idx)
    rel_idx = np.asarray(rel_idx)
    gov_idx = np.asarray(gov_idx)
    W_self = np.asarray(W_self)
    b_self = np.asarray(b_self)
    W_rel = np.asarray(W_rel)
    b_rel = np.asarray(b_rel)
    W_ff = np.asarray(W_ff)
    b_ff = np.asarray(b_ff)
    assert x.shape == (N, D) and W_rel.shape == (L, TWO_R, D, D)

    key = (dep_idx.tobytes(), rel_idx.tobytes(), gov_idx.tobytes())
    if key in _CACHE:
        nc, plan = _CACHE[key]
    else:
        plan = _plan(dep_idx, rel_idx, gov_idx)
        nc = _build(
            plan["MT"], plan["tile_slot"], plan["NMSG"], plan["R1M"], plan["OVER"],
            plan["SEG"], plan["SENDH"], plan["SENDALL"], plan["J2M"], plan["J2O"],
            plan["NCHUNK"],
        )
        _CACHE.clear()
        _CACHE[key] = (nc, plan)

    in_maps = _in_maps(plan, x, W_self, b_self, W_rel, b_rel, W_ff, b_ff)
    res = run_bass_kernel_spmd(nc, in_maps, list(range(NCORES)))
    LAST_EXEC_TIME_NS = res.exec_time_ns
    LAST_RESULTS = res
    out = np.concatenate([res.results[c]["y"] for c in range(NCORES)], axis=0)
    return out.astype(np.float32)
